# revision 1
# baseline (speedup 1.0000x reference)
"""GCN (GCNConv + 3-layer MLP + log_softmax) on 8 Trainium2 NeuronCores.

Strategy (pull-mode message passing):
  - Nodes are sharded 8 ways by destination; each core owns 12500 dst nodes
    (padded to 12544 = 98 tiles of 128).
  - Every core computes the full transformed feature table h = x @ W_gcn
    ([100352, 64] fp32, rows padded) into its own DRAM — replicating this
    small matmul is cheaper than an AllGather of h.
  - Edges (incl. self-loops) are partitioned by dst shard on the host,
    sorted by (dst tile, src group, src), padded to 128-edge chunks.
  - Per 128-edge chunk the core gathers h[src] rows with dma_gather
    (256 B/row), builds a scaled one-hot matrix S[e, j] = norm[e] *
    (dst_local[e] == j) with one fused tensor_scalar op, and accumulates
    aggT[64, 128] += msgs.T @ S on the tensor engine (PSUM).
  - The MLP runs in feature-major (transposed) layout so all biases are
    per-partition activation biases; the last matmul flips back to
    node-major and log_softmax finishes on [128, 4] tiles.
"""

import os
import sys

import numpy as np

sys.path.insert(0, "/opt/trn_rl_repo")

N = 100000
F = 256
H = 64
NCLS = 4
NCORES = 8
SHARD = 12500
SPAD = 12544          # 98 * 128
NT = SPAD // 128      # 98 dst tiles per core
NPAD = SPAD * NCORES  # 100352
NG = 4
GSZ = NPAD // NG      # 25088 rows per src group (< 2**15 for int16 idx)
TSB1 = 8              # phase-1 tiles per superblock; 1024-row blocks align
                      # with the half-table boundary (50176 = 49*1024)
TSB3 = 6              # phase-3 dst tiles per superblock


def _host_prep(edge_index):
    """Partition/sort/pad edges; returns per-core device arrays + meta."""
    src = np.asarray(edge_index[0]).astype(np.int64)
    dst = np.asarray(edge_index[1]).astype(np.int64)
    deg = np.bincount(dst, minlength=N).astype(np.float64) + 1.0
    dinv = 1.0 / np.sqrt(deg)

    loop = np.arange(N, dtype=np.int64)
    srcA = np.concatenate([src, loop])
    dstA = np.concatenate([dst, loop])
    norm = (dinv[srcA] * dinv[dstA]).astype(np.float32)

    core = dstA // SHARD
    dl = dstA - core * SHARD
    tl = dl >> 7
    dloc = (dl & 127).astype(np.float32)
    srcp = (srcA // SHARD) * SPAD + (srcA % SHARD)   # padded global src id
    # h_all rows are stored partition-major per phase-1 superblock (so the
    # h write DMA is contiguous): node srcp lives at h_all row perm(srcp).
    blk = TSB1 * 128
    b = srcp // blk
    r = srcp - b * blk
    srcp = b * blk + (r % 128) * TSB1 + r // 128
    # groups: (half-table, row parity) — parity spreads each core's
    # self-loop band over both groups of its half, so the max-over-cores
    # chunk count is not inflated by the +128 self-loop concentration
    half = srcp // (2 * GSZ)
    w = srcp - half * (2 * GSZ)
    grp = half * 2 + (w & 1)
    idx16 = (w >> 1).astype(np.int16)

    key = ((core * NT + tl) * NG + grp)
    order = np.argsort(key * np.int64(NPAD) + srcp, kind="stable")
    key_s = key[order]
    idx_s = idx16[order]
    dloc_s = dloc[order]
    norm_s = norm[order]

    cnt = np.bincount(key, minlength=NCORES * NT * NG).reshape(NCORES, NT, NG)
    C = ((cnt.max(axis=0) + 127) // 128).astype(np.int64)      # [NT, NG] chunks
    starts = np.zeros(NCORES * NT * NG + 1, dtype=np.int64)
    np.cumsum(cnt.reshape(-1), out=starts[1:])

    # superblock partition of the 98 tiles
    sbs = [list(range(s, min(s + TSB3, NT))) for s in range(0, NT, TSB3)]
    # stream layout: for sb: for g: for t in sb: C[t,g] chunks of 128 edges
    col_of = np.zeros((NT, NG), dtype=np.int64)   # chunk column of (t, g)
    sb_meta = []
    col = 0
    for tiles in sbs:
        colbase = col
        Ls = []
        goffs = []
        for g in range(NG):
            goffs.append(col - colbase)
            for t in tiles:
                col_of[t, g] = col
                col += C[t, g]
            Ls.append(int(128 * sum(C[t, g] for t in tiles)))
        sb_meta.append(dict(tiles=tiles, colbase=int(colbase),
                            totc=int(col - colbase), L=Ls, goff=goffs))
    TOTC = int(col)
    TOT = TOTC * 128

    idx_streams, dloc_streams, norm_streams = [], [], []
    for c in range(NCORES):
        si = np.zeros(TOT, dtype=np.int16)
        sd = np.full(TOT, -1.0, dtype=np.float32)
        sn = np.zeros(TOT, dtype=np.float32)
        for t in range(NT):
            for g in range(NG):
                k = (c * NT + t) * NG + g
                n = cnt[c, t, g]
                if n == 0:
                    continue
                a = starts[k]
                o = col_of[t, g] * 128
                si[o:o + n] = idx_s[a:a + n]
                sd[o:o + n] = dloc_s[a:a + n]
                sn[o:o + n] = norm_s[a:a + n]
        idx_streams.append(np.tile(si.reshape(-1, 16).T, (8, 1)))      # [128, TOT/16]
        dloc_streams.append(np.ascontiguousarray(sd.reshape(-1, 128).T))  # [128, TOTC]
        norm_streams.append(np.ascontiguousarray(sn.reshape(-1, 128).T))
    meta = dict(C=C, sb_meta=sb_meta, TOTC=TOTC, TOT=TOT)
    return idx_streams, dloc_streams, norm_streams, meta


def _build_nc(meta):
    import concourse.bacc as bacc
    import concourse.mybir as mybir
    import concourse.tile as tile
    from concourse import library_config

    f32 = mybir.dt.float32
    i16 = mybir.dt.int16
    AF = mybir.ActivationFunctionType
    ALU = mybir.AluOpType
    TOTC, TOT = meta["TOTC"], meta["TOT"]
    C, sb_meta = meta["C"], meta["sb_meta"]

    nc = bacc.Bacc("TRN2")
    xT = nc.dram_tensor("xT", [F, NPAD], f32, kind="ExternalInput")
    wg = nc.dram_tensor("wg", [F, H], f32, kind="ExternalInput")
    w1 = nc.dram_tensor("w1", [64, 32], f32, kind="ExternalInput")
    w2 = nc.dram_tensor("w2", [32, 16], f32, kind="ExternalInput")
    w3 = nc.dram_tensor("w3", [16, 4], f32, kind="ExternalInput")
    bg = nc.dram_tensor("bg", [64, 1], f32, kind="ExternalInput")
    b1 = nc.dram_tensor("b1", [32, 1], f32, kind="ExternalInput")
    b2 = nc.dram_tensor("b2", [16, 1], f32, kind="ExternalInput")
    b3r = nc.dram_tensor("b3r", [1, 4], f32, kind="ExternalInput")
    iotam = nc.dram_tensor("iotam", [128, 128], f32, kind="ExternalInput")
    onesr = nc.dram_tensor("onesr", [1, 128], f32, kind="ExternalInput")
    idxT = nc.dram_tensor("idx", [128, TOT // 16], i16, kind="ExternalInput")
    dlocT = nc.dram_tensor("dloc", [128, TOTC], f32, kind="ExternalInput")
    nrmT = nc.dram_tensor("nrm", [128, TOTC], f32, kind="ExternalInput")
    outT = nc.dram_tensor("out", [SPAD, NCLS], f32, kind="ExternalOutput")

    NT1 = NPAD // 128  # 784 phase-1 tiles
    sb1 = [list(range(s, min(s + TSB1, NT1))) for s in range(0, NT1, TSB1)]
    # per-pass (groups 0-1 / groups 2-3) chunk-count maxima for tile sizing
    maxc0 = max(m["goff"][2] for m in sb_meta)
    maxc1 = max(m["totc"] - m["goff"][2] for m in sb_meta)
    maxc = max(maxc0, maxc1)

    with tile.TileContext(nc) as tc:
        with tc.tile_pool(name="const", bufs=1) as cp, \
             tc.tile_pool(name="dram", bufs=1, space="DRAM") as dram:
            h01 = dram.tile([2 * GSZ, H], f32, tag="h01")
            h23 = dram.tile([2 * GSZ, H], f32, tag="h23")
            nc.gpsimd.load_library(library_config.mlp)

            wg0 = cp.tile([128, H], f32, tag="wg0")
            wg1 = cp.tile([128, H], f32, tag="wg1")
            nc.sync.dma_start(wg0[:], wg[0:128, :])
            nc.sync.dma_start(wg1[:], wg[128:256, :])
            w1s = cp.tile([64, 32], f32, tag="w1s")
            w2s = cp.tile([32, 16], f32, tag="w2s")
            w3s = cp.tile([16, 4], f32, tag="w3s")
            bgs = cp.tile([64, 1], f32, tag="bgs")
            b1s = cp.tile([32, 1], f32, tag="b1s")
            b2s = cp.tile([16, 1], f32, tag="b2s")
            b3s = cp.tile([1, 4], f32, tag="b3s")
            iots = cp.tile([128, 128], f32, tag="iots")
            ones = cp.tile([1, 128], f32, tag="ones")
            for t_, d_ in ((w1s, w1), (w2s, w2), (w3s, w3), (bgs, bg),
                           (b1s, b1), (b2s, b2), (b3s, b3r), (iots, iotam),
                           (ones, onesr)):
                nc.sync.dma_start(t_[:], d_[:, :])

            # All pools stay open so phase 1 overlaps the pass-0 gathers
            # (closing/reusing SBUF zones would add false dependencies).
            with tc.tile_pool(name="p1", bufs=2) as p1p, \
                 tc.tile_pool(name="ps1", bufs=2, space="PSUM") as ps1, \
                 tc.tile_pool(name="p3", bufs=3) as p3p, \
                 tc.tile_pool(name="gb", bufs=3) as gbp, \
                 tc.tile_pool(name="sp", bufs=6) as sp, \
                 tc.tile_pool(name="ep", bufs=3) as ep, \
                 tc.tile_pool(name="oa", bufs=1) as oap, \
                 tc.tile_pool(name="agg", bufs=3, space="PSUM") as aggp, \
                 tc.tile_pool(name="mlp", bufs=3, space="PSUM") as mlpp:
                # -------- phase 1: h = x @ W_gcn, halves written in order ---
                nhalf = len(sb1) // 2
                for bi, tiles in enumerate(sb1):
                    T = len(tiles)
                    t0 = tiles[0]
                    xt0 = p1p.tile([128, TSB1 * 128], f32, tag="xt0")
                    xt1 = p1p.tile([128, TSB1 * 128], f32, tag="xt1")
                    nc.sync.dma_start(
                        xt0[:, :T * 128], xT[0:128, t0 * 128:(t0 + T) * 128])
                    nc.sync.dma_start(
                        xt1[:, :T * 128], xT[128:256, t0 * 128:(t0 + T) * 128])
                    hsb = p1p.tile([128, TSB1, H], f32, tag="hsb")
                    for i in range(T):
                        ps = ps1.tile([128, H], f32, tag="hps")
                        nc.tensor.matmul(ps[:], xt0[:, i * 128:(i + 1) * 128],
                                         wg0[:], start=True, stop=False)
                        nc.tensor.matmul(ps[:], xt1[:, i * 128:(i + 1) * 128],
                                         wg1[:], start=False, stop=True)
                        nc.vector.tensor_copy(hsb[:, i, :], ps[:])
                    hP = h01 if bi < nhalf else h23
                    r0 = (bi if bi < nhalf else bi - nhalf) * TSB1 * 128
                    # partition-major row order -> contiguous 2 KB runs
                    nc.sync.dma_start(
                        hP[r0:r0 + T * 128, :]
                        .rearrange("(p t) f -> p t f", p=128),
                        hsb[:, :T, :])

                # -------- phase 3: two passes (groups 0-1, then 2-3) --------
                outacc = oap.tile([128, NT, NCLS], f32, tag="outacc")
                accT = oap.tile([64, NT * 128], f32, tag="accT")
                for pas in (0, 1):
                    hP = h01 if pas == 0 else h23
                    gl, gh = 2 * pas, 2 * pas + 2
                    for m in sb_meta:
                        tiles = m["tiles"]
                        pco = m["goff"][gl]                  # pass col offset
                        pend = m["totc"] if pas else m["goff"][2]
                        ptc = pend - pco                     # pass chunk count
                        cb = m["colbase"] + pco              # global col base
                        idxsb = p3p.tile([128, maxc * 8], i16, tag="idx")
                        nc.sync.dma_start(idxsb[:, :ptc * 8],
                                          idxT[:, cb * 8:(cb + ptc) * 8])
                        dlsb = p3p.tile([128, maxc], f32, tag="dl")
                        nrsb = p3p.tile([128, maxc], f32, tag="nr")
                        nc.sync.dma_start(dlsb[:, :ptc],
                                          dlocT[:, cb:cb + ptc])
                        nc.sync.dma_start(nrsb[:, :ptc],
                                          nrmT[:, cb:cb + ptc])
                        gbuf = gbp.tile([128, maxc, H], f32, tag="gbuf")
                        for g in range(gl, gh):
                            L = m["L"][g]
                            go = m["goff"][g] - pco
                            # SWDGE ring caps one gather at ~1024 idxs
                            for k in range(0, L, 1024):
                                ni = min(1024, L - k)
                                c0 = go + k // 128
                                hV = hP[:].rearrange(
                                    "(r two) f -> two r f", two=2)[g - gl]
                                nc.gpsimd.dma_gather(
                                    gbuf[:, c0:c0 + ni // 128, :],
                                    hV,
                                    idxsb[:, c0 * 8:(c0 + ni // 128) * 8],
                                    ni, ni, H, elem_step=2 * H)
                        for ti, t in enumerate(tiles):
                            agg = aggp.tile([64, 128], f32, tag="agg")
                            nch = int(C[t, gl:gh].sum())
                            done = 0
                            for g in range(gl, gh):
                                base = (m["goff"][g] - pco) + int(
                                    sum(C[tt, g] for tt in tiles[:ti]))
                                for j in range(int(C[t, g])):
                                    pos = base + j
                                    S = sp.tile([128, 128], f32, tag="S")
                                    nc.vector.tensor_scalar(
                                        S[:], iots[:], dlsb[:, pos:pos + 1],
                                        nrsb[:, pos:pos + 1],
                                        op0=ALU.is_equal, op1=ALU.mult)
                                    nc.tensor.matmul(
                                        agg[:], gbuf[:, pos, :], S[:],
                                        start=(done == 0),
                                        stop=(done == nch - 1))
                                    done += 1
                            if pas == 0:
                                if nch == 0:
                                    nc.vector.memset(
                                        accT[:, t * 128:(t + 1) * 128], 0.0)
                                else:
                                    nc.vector.tensor_copy(
                                        accT[:, t * 128:(t + 1) * 128], agg[:])
                                continue
                            t0p = ep.tile([64, 128], f32, tag="t0p")
                            nc.vector.tensor_add(
                                t0p[:], accT[:, t * 128:(t + 1) * 128], agg[:])
                            t0s = ep.tile([64, 128], f32, tag="t0")
                            nc.scalar.activation(t0s[:], t0p[:], AF.Relu,
                                                 bias=bgs[:])
                            pm1 = mlpp.tile([32, 128], f32, tag="pm")
                            nc.tensor.matmul(pm1[:], w1s[:], t0s[:],
                                             start=True, stop=True)
                            t1s = ep.tile([32, 128], f32, tag="t1")
                            nc.scalar.activation(t1s[:], pm1[:], AF.Relu,
                                                 bias=b1s[:])
                            pm2 = mlpp.tile([16, 128], f32, tag="pm")
                            nc.tensor.matmul(pm2[:], w2s[:], t1s[:],
                                             start=True, stop=True)
                            t2s = ep.tile([16, 128], f32, tag="t2")
                            nc.scalar.activation(t2s[:], pm2[:], AF.Relu,
                                                 bias=b2s[:])
                            pm3 = mlpp.tile([128, NCLS], f32, tag="pm")
                            nc.tensor.matmul(pm3[:], t2s[:], w3s[:],
                                             start=True, stop=False)
                            nc.tensor.matmul(pm3[:], ones[:], b3s[:],
                                             start=False, stop=True)
                            nmax = ep.tile([128, 1], f32, tag="nmax")
                            nc.vector.tensor_reduce(nmax[:], pm3[:],
                                                    axis=mybir.AxisListType.X,
                                                    op=ALU.max, negate=True)
                            esb = ep.tile([128, NCLS], f32, tag="esb")
                            ssum = ep.tile([128, 1], f32, tag="ssum")
                            nc.scalar.activation(esb[:], pm3[:], AF.Exp,
                                                 bias=nmax[:], accum_out=ssum[:])
                            lsb = ep.tile([128, 1], f32, tag="lsb")
                            nc.scalar.activation(lsb[:], ssum[:], AF.Ln)
                            nc.vector.tensor_scalar(
                                outacc[:, t, :], pm3[:], nmax[:], lsb[:],
                                op0=ALU.add, op1=ALU.subtract)
                nc.sync.dma_start(
                    outT[:, :].rearrange("(t p) c -> p t c", p=128),
                    outacc[:])
    nc.compile()
    return nc


def kernel(x, edge_index, W_gcn, b_gcn, W1, b1, W2, b2, W3, b3,
           _trace=False):
    from concourse.bass_utils import run_bass_kernel_spmd

    x = np.asarray(x, dtype=np.float32)
    idx_streams, dloc_streams, norm_streams, meta = _host_prep(edge_index)
    nc = _build_nc(meta)

    xTp = np.zeros((F, NPAD), dtype=np.float32)
    xt = np.ascontiguousarray(x.T)
    for c in range(NCORES):
        xTp[:, c * SPAD:c * SPAD + SHARD] = xt[:, c * SHARD:(c + 1) * SHARD]
    common = {
        "xT": xTp,
        "wg": np.asarray(W_gcn, np.float32),
        "w1": np.asarray(W1, np.float32),
        "w2": np.asarray(W2, np.float32),
        "w3": np.asarray(W3, np.float32),
        "bg": np.asarray(b_gcn, np.float32).reshape(64, 1),
        "b1": np.asarray(b1, np.float32).reshape(32, 1),
        "b2": np.asarray(b2, np.float32).reshape(16, 1),
        "b3r": np.asarray(b3, np.float32).reshape(1, 4),
        "iotam": np.tile(np.arange(128, dtype=np.float32), (128, 1)),
        "onesr": np.ones((1, 128), dtype=np.float32),
    }
    in_maps = []
    for c in range(NCORES):
        m = dict(common)
        m["idx"] = idx_streams[c]
        m["dloc"] = dloc_streams[c]
        m["nrm"] = norm_streams[c]
        in_maps.append(m)

    res = run_bass_kernel_spmd(nc, in_maps, core_ids=list(range(NCORES)),
                               trace=_trace)
    out = np.concatenate(
        [res.results[c]["out"][:SHARD] for c in range(NCORES)], axis=0)
    if _trace:
        kernel.last_exec_time_ns = res.exec_time_ns
    return out


kernel.last_exec_time_ns = None



# revision 5
# speedup vs baseline: 1.8578x; 1.8578x over previous
"""GCN (GCNConv + 3-layer MLP + log_softmax) on 8 Trainium2 NeuronCores.

Strategy (pull-mode message passing):
  - Nodes are sharded 8 ways by destination; each core owns 12500 dst nodes
    (padded to 12544 = 98 tiles of 128).
  - Every core computes the full transformed feature table
    h = (dinv * x) @ W_gcn ([100352, 64] bf16, rows padded, dinv[src]
    folded into x on the host) into its own DRAM — replicating this small
    matmul is cheaper than an AllGather of h.
  - Edges (incl. self-loops) are partitioned by dst shard on the host,
    sorted by (dst tile, src group, src), padded to 128-edge chunks.
    Groups = (table half, row parity): the bf16 table is gathered in
    256-byte units covering a PAIR of rows, so a group fixes which half
    of the gathered pair is the live row.
  - Per chunk the core gathers h row-pairs with dma_gather (256 B/row
    pair); gathers round-robin across all 4 SWDGE queues so descriptor
    generation uses all 8 Q7 cores (4x the single-queue rate).
  - A scaled one-hot S[e, j] = dinv[dst_e] * (dst_local[e] == j) (bf16)
    is built with one fused tensor_scalar op, and the tensor engine
    accumulates aggT[64, 128] += msgs.T @ S in PSUM.
  - The MLP runs in feature-major (transposed) layout so all biases are
    per-partition activation biases; the last matmul flips back to
    node-major and log_softmax finishes on [128, 4] tiles.
"""

import os
import sys

import numpy as np

sys.path.insert(0, "/opt/trn_rl_repo")

N = 100000
F = 256
H = 64
NCLS = 4
NCORES = 8
SHARD = 12500
SPAD = 12544          # 98 * 128
NT = SPAD // 128      # 98 dst tiles per core
NPAD = SPAD * NCORES  # 100352
NG = 4
GSZ = NPAD // NG      # 25088 row-pairs per group half (< 2**15 for int16)
TSB1 = 8              # phase-1 tiles per superblock; 1024-row blocks align
                      # with the half-table boundary (50176 = 49*1024)
TSB3 = 6              # phase-3 dst tiles per superblock
GCALL = 1024          # idxs per dma_gather call (SWDGE ring cap)


def _host_prep(edge_index):
    """Partition/sort/pad edges; returns per-core device arrays + meta."""
    src = np.asarray(edge_index[0]).astype(np.int64)
    dst = np.asarray(edge_index[1]).astype(np.int64)
    deg = np.bincount(dst, minlength=N).astype(np.float64) + 1.0
    dinv = 1.0 / np.sqrt(deg)

    loop = np.arange(N, dtype=np.int64)
    srcA = np.concatenate([src, loop])
    dstA = np.concatenate([dst, loop])
    nrm = dinv[dstA].astype(np.float32)   # dinv[src] is folded into x

    core = dstA // SHARD
    dl = dstA - core * SHARD
    tl = dl >> 7
    dloc = (dl & 127).astype(np.float32)
    srcp = (srcA // SHARD) * SPAD + (srcA % SHARD)   # padded global src id
    # h rows are stored partition-major per phase-1 superblock (so the
    # h write DMA is contiguous): node srcp lives at h row perm(srcp).
    blk = TSB1 * 128
    b = srcp // blk
    r = srcp - b * blk
    srcp = b * blk + (r % 128) * TSB1 + r // 128
    # groups: (half-table, row parity) — parity selects which half of the
    # gathered 256-byte row pair is live; it also spreads each core's
    # self-loop band over both groups of its half.
    half = srcp // (2 * GSZ)
    w = srcp - half * (2 * GSZ)
    grp = half * 2 + (w & 1)
    idx16 = (w >> 1).astype(np.int16)     # pair index within the half

    key = ((core * NT + tl) * NG + grp)
    order = np.argsort(key * np.int64(NPAD) + srcp, kind="stable")
    key_s = key[order]
    idx_s = idx16[order]
    dloc_s = dloc[order]
    nrm_s = nrm[order]

    cnt = np.bincount(key, minlength=NCORES * NT * NG).reshape(NCORES, NT, NG)
    C = ((cnt.max(axis=0) + 127) // 128).astype(np.int64)      # [NT, NG] chunks
    starts = np.zeros(NCORES * NT * NG + 1, dtype=np.int64)
    np.cumsum(cnt.reshape(-1), out=starts[1:])

    # superblock partition of the 98 tiles
    sbs = [list(range(s, min(s + TSB3, NT))) for s in range(0, NT, TSB3)]
    # stream layout: for sb: for g: for t in sb: C[t,g] chunks of 128 edges
    col_of = np.zeros((NT, NG), dtype=np.int64)   # chunk column of (t, g)
    sb_meta = []
    col = 0
    for tiles in sbs:
        colbase = col
        Ls = []
        goffs = []
        for g in range(NG):
            goffs.append(col - colbase)
            for t in tiles:
                col_of[t, g] = col
                col += C[t, g]
            Ls.append(int(128 * sum(C[t, g] for t in tiles)))
        sb_meta.append(dict(tiles=tiles, colbase=int(colbase),
                            totc=int(col - colbase), L=Ls, goff=goffs))
    TOTC = int(col)
    TOT = TOTC * 128

    import ml_dtypes
    bf16 = ml_dtypes.bfloat16
    idx_streams, dloc_streams, nrm_streams = [], [], []
    for c in range(NCORES):
        si = np.zeros(TOT, dtype=np.int16)
        sd = np.full(TOT, -1.0, dtype=np.float32)
        sn = np.zeros(TOT, dtype=np.float32)
        for t in range(NT):
            for g in range(NG):
                k = (c * NT + t) * NG + g
                n = cnt[c, t, g]
                if n == 0:
                    continue
                a = starts[k]
                o = col_of[t, g] * 128
                si[o:o + n] = idx_s[a:a + n]
                sd[o:o + n] = dloc_s[a:a + n]
                sn[o:o + n] = nrm_s[a:a + n]
        idx_streams.append(np.tile(si.reshape(-1, 16).T, (8, 1)))      # [128, TOT/16]
        dloc_streams.append(np.ascontiguousarray(sd.reshape(-1, 128).T))  # [128, TOTC]
        nrm_streams.append(np.ascontiguousarray(sn.reshape(-1, 128).T))
    meta = dict(C=C, sb_meta=sb_meta, TOTC=TOTC, TOT=TOT)
    return idx_streams, dloc_streams, nrm_streams, meta, dinv


def _build_nc(meta):
    import concourse.bacc as bacc
    import concourse.mybir as mybir
    import concourse.tile as tile
    from concourse import library_config

    f32 = mybir.dt.float32
    bf16 = mybir.dt.bfloat16
    i16 = mybir.dt.int16
    AF = mybir.ActivationFunctionType
    ALU = mybir.AluOpType
    TOTC, TOT = meta["TOTC"], meta["TOT"]
    C, sb_meta = meta["C"], meta["sb_meta"]

    nc = bacc.Bacc("TRN2", num_swdge_queues=4)
    xT = nc.dram_tensor("xT", [F, NPAD], bf16, kind="ExternalInput")
    wg = nc.dram_tensor("wg", [F, H], bf16, kind="ExternalInput")
    w1 = nc.dram_tensor("w1", [64, 32], bf16, kind="ExternalInput")
    w2 = nc.dram_tensor("w2", [32, 16], bf16, kind="ExternalInput")
    w3 = nc.dram_tensor("w3", [16, 4], bf16, kind="ExternalInput")
    bg = nc.dram_tensor("bg", [64, 1], f32, kind="ExternalInput")
    b1 = nc.dram_tensor("b1", [32, 1], f32, kind="ExternalInput")
    b2 = nc.dram_tensor("b2", [16, 1], f32, kind="ExternalInput")
    b3r = nc.dram_tensor("b3r", [1, 4], bf16, kind="ExternalInput")
    iotam = nc.dram_tensor("iotam", [128, 128], bf16, kind="ExternalInput")
    onesr = nc.dram_tensor("onesr", [1, 128], bf16, kind="ExternalInput")
    idxT = nc.dram_tensor("idx", [128, TOT // 16], i16, kind="ExternalInput")
    dlocT = nc.dram_tensor("dloc", [128, TOTC], f32, kind="ExternalInput")
    nrmT = nc.dram_tensor("nrm", [128, TOTC], f32, kind="ExternalInput")
    outT = nc.dram_tensor("out", [SPAD, NCLS], f32, kind="ExternalOutput")

    NT1 = NPAD // 128  # 784 phase-1 tiles
    sb1 = [list(range(s, min(s + TSB1, NT1))) for s in range(0, NT1, TSB1)]
    # per-pass (groups 0-1 / groups 2-3) chunk-count maxima for tile sizing
    maxc0 = max(m["goff"][2] for m in sb_meta)
    maxc1 = max(m["totc"] - m["goff"][2] for m in sb_meta)
    maxc = max(maxc0, maxc1)

    with tile.TileContext(nc) as tc:
        with tc.tile_pool(name="const", bufs=1) as cp, \
             tc.tile_pool(name="dram", bufs=1, space="DRAM") as dram:
            # each half stored as row pairs: [25088 pairs, 128] bf16
            h01 = dram.tile([GSZ, 2 * H], bf16, tag="h01")
            h23 = dram.tile([GSZ, 2 * H], bf16, tag="h23")
            nc.gpsimd.load_library(library_config.mlp)

            wg0 = cp.tile([128, H], bf16, tag="wg0")
            wg1 = cp.tile([128, H], bf16, tag="wg1")
            nc.sync.dma_start(wg0[:], wg[0:128, :])
            nc.sync.dma_start(wg1[:], wg[128:256, :])
            w1s = cp.tile([64, 32], bf16, tag="w1s")
            w2s = cp.tile([32, 16], bf16, tag="w2s")
            w3s = cp.tile([16, 4], bf16, tag="w3s")
            bgs = cp.tile([64, 1], f32, tag="bgs")
            b1s = cp.tile([32, 1], f32, tag="b1s")
            b2s = cp.tile([16, 1], f32, tag="b2s")
            b3s = cp.tile([1, 4], bf16, tag="b3s")
            iots = cp.tile([128, 128], bf16, tag="iots")
            ones = cp.tile([1, 128], bf16, tag="ones")
            for t_, d_ in ((w1s, w1), (w2s, w2), (w3s, w3), (bgs, bg),
                           (b1s, b1), (b2s, b2), (b3s, b3r), (iots, iotam),
                           (ones, onesr)):
                nc.sync.dma_start(t_[:], d_[:, :])

            # All pools stay open so phase 1 overlaps the pass-0 gathers
            # (closing/reusing SBUF zones would add false dependencies).
            with tc.tile_pool(name="p1", bufs=2) as p1p, \
                 tc.tile_pool(name="ps1", bufs=2, space="PSUM") as ps1, \
                 tc.tile_pool(name="p3", bufs=3) as p3p, \
                 tc.tile_pool(name="gb", bufs=3) as gbp, \
                 tc.tile_pool(name="sp", bufs=6) as sp, \
                 tc.tile_pool(name="ep", bufs=3) as ep, \
                 tc.tile_pool(name="oa", bufs=1) as oap, \
                 tc.tile_pool(name="agg", bufs=3, space="PSUM") as aggp, \
                 tc.tile_pool(name="mlp", bufs=3, space="PSUM") as mlpp:
                # -------- phase 1: h = x @ W_gcn, halves written in order ---
                nhalf = len(sb1) // 2
                for bi, tiles in enumerate(sb1):
                    T = len(tiles)
                    t0 = tiles[0]
                    xt0 = p1p.tile([128, TSB1 * 128], bf16, tag="xt0")
                    xt1 = p1p.tile([128, TSB1 * 128], bf16, tag="xt1")
                    nc.sync.dma_start(
                        xt0[:, :T * 128], xT[0:128, t0 * 128:(t0 + T) * 128])
                    nc.sync.dma_start(
                        xt1[:, :T * 128], xT[128:256, t0 * 128:(t0 + T) * 128])
                    hsb = p1p.tile([128, TSB1 * H], bf16, tag="hsb")
                    for i in range(T):
                        ps = ps1.tile([128, H], f32, tag="hps")
                        nc.tensor.matmul(ps[:], xt0[:, i * 128:(i + 1) * 128],
                                         wg0[:], start=True, stop=False)
                        nc.tensor.matmul(ps[:], xt1[:, i * 128:(i + 1) * 128],
                                         wg1[:], start=False, stop=True)
                        nc.vector.tensor_copy(hsb[:, i * H:(i + 1) * H], ps[:])
                    hP = h01 if bi < nhalf else h23
                    r0 = (bi if bi < nhalf else bi - nhalf) * TSB1 * 128
                    # partition-major row order -> per-partition contiguous
                    # 1 KB runs: partition p holds rows r0+p*T..r0+p*T+T-1,
                    # i.e. pair rows (r0//2)+p*(T//2).. of the [GSZ,128] table
                    nc.sync.dma_start(
                        hP[r0 // 2:(r0 + T * 128) // 2, :]
                        .rearrange("(p q) f -> p q f", p=128),
                        hsb[:].rearrange("p (q f) -> p q f", q=T // 2))

                # -------- phase 3: two passes (half 0, then half 1) --------
                outacc = oap.tile([128, NT, NCLS], f32, tag="outacc")
                accT = oap.tile([64, NT * 128], f32, tag="accT")
                gq = 0
                for pas in (0, 1):
                    hP = h01 if pas == 0 else h23
                    gl, gh = 2 * pas, 2 * pas + 2
                    for m in sb_meta:
                        tiles = m["tiles"]
                        pco = m["goff"][gl]                  # pass col offset
                        pend = m["totc"] if pas else m["goff"][2]
                        ptc = pend - pco                     # pass chunk count
                        cb = m["colbase"] + pco              # global col base
                        idxsb = p3p.tile([128, maxc * 8], i16, tag="idx")
                        nc.sync.dma_start(idxsb[:, :ptc * 8],
                                          idxT[:, cb * 8:(cb + ptc) * 8])
                        dlsb = p3p.tile([128, maxc], f32, tag="dl")
                        nrsb = p3p.tile([128, maxc], f32, tag="nr")
                        nc.sync.dma_start(dlsb[:, :ptc],
                                          dlocT[:, cb:cb + ptc])
                        nc.sync.dma_start(nrsb[:, :ptc],
                                          nrmT[:, cb:cb + ptc])
                        gbuf = gbp.tile([128, maxc, 2 * H], bf16, tag="gbuf")
                        for g in range(gl, gh):
                            L = m["L"][g]
                            go = m["goff"][g] - pco
                            for k in range(0, L, GCALL):
                                ni = min(GCALL, L - k)
                                c0 = go + k // 128
                                nc.gpsimd.dma_gather(
                                    gbuf[:, c0:c0 + ni // 128, :],
                                    hP[:],
                                    idxsb[:, c0 * 8:(c0 + ni // 128) * 8],
                                    ni, ni, 2 * H, queue_num=gq % 4)
                                gq += 1
                        for ti, t in enumerate(tiles):
                            agg = aggp.tile([64, 128], f32, tag="agg")
                            nch = int(C[t, gl:gh].sum())
                            done = 0
                            for g in range(gl, gh):
                                base = (m["goff"][g] - pco) + int(
                                    sum(C[tt, g] for tt in tiles[:ti]))
                                par = g & 1
                                for j in range(int(C[t, g])):
                                    pos = base + j
                                    S = sp.tile([128, 128], bf16, tag="S")
                                    nc.vector.tensor_scalar(
                                        S[:], iots[:], dlsb[:, pos:pos + 1],
                                        nrsb[:, pos:pos + 1],
                                        op0=ALU.is_equal, op1=ALU.mult)
                                    nc.tensor.matmul(
                                        agg[:],
                                        gbuf[:, pos, par * H:(par + 1) * H],
                                        S[:],
                                        start=(done == 0),
                                        stop=(done == nch - 1))
                                    done += 1
                            if pas == 0:
                                if nch == 0:
                                    nc.vector.memset(
                                        accT[:, t * 128:(t + 1) * 128], 0.0)
                                else:
                                    nc.vector.tensor_copy(
                                        accT[:, t * 128:(t + 1) * 128], agg[:])
                                continue
                            t0p = ep.tile([64, 128], f32, tag="t0p")
                            nc.vector.tensor_add(
                                t0p[:], accT[:, t * 128:(t + 1) * 128], agg[:])
                            t0s = ep.tile([64, 128], bf16, tag="t0")
                            nc.scalar.activation(t0s[:], t0p[:], AF.Relu,
                                                 bias=bgs[:])
                            pm1 = mlpp.tile([32, 128], f32, tag="pm")
                            nc.tensor.matmul(pm1[:], w1s[:], t0s[:],
                                             start=True, stop=True)
                            t1s = ep.tile([32, 128], bf16, tag="t1")
                            nc.scalar.activation(t1s[:], pm1[:], AF.Relu,
                                                 bias=b1s[:])
                            pm2 = mlpp.tile([16, 128], f32, tag="pm")
                            nc.tensor.matmul(pm2[:], w2s[:], t1s[:],
                                             start=True, stop=True)
                            t2s = ep.tile([16, 128], bf16, tag="t2")
                            nc.scalar.activation(t2s[:], pm2[:], AF.Relu,
                                                 bias=b2s[:])
                            pm3 = mlpp.tile([128, NCLS], f32, tag="pm")
                            nc.tensor.matmul(pm3[:], t2s[:], w3s[:],
                                             start=True, stop=False)
                            nc.tensor.matmul(pm3[:], ones[:], b3s[:],
                                             start=False, stop=True)
                            nmax = ep.tile([128, 1], f32, tag="nmax")
                            nc.vector.tensor_reduce(nmax[:], pm3[:],
                                                    axis=mybir.AxisListType.X,
                                                    op=ALU.max, negate=True)
                            esb = ep.tile([128, NCLS], f32, tag="esb")
                            ssum = ep.tile([128, 1], f32, tag="ssum")
                            nc.scalar.activation(esb[:], pm3[:], AF.Exp,
                                                 bias=nmax[:], accum_out=ssum[:])
                            lsb = ep.tile([128, 1], f32, tag="lsb")
                            nc.scalar.activation(lsb[:], ssum[:], AF.Ln)
                            nc.vector.tensor_scalar(
                                outacc[:, t, :], pm3[:], nmax[:], lsb[:],
                                op0=ALU.add, op1=ALU.subtract)
                nc.sync.dma_start(
                    outT[:, :].rearrange("(t p) c -> p t c", p=128),
                    outacc[:])
    nc.compile()
    return nc


def kernel(x, edge_index, W_gcn, b_gcn, W1, b1, W2, b2, W3, b3,
           _trace=False):
    import ml_dtypes
    from concourse.bass_utils import run_bass_kernel_spmd

    bf16 = ml_dtypes.bfloat16
    x = np.asarray(x, dtype=np.float32)
    idx_streams, dloc_streams, nrm_streams, meta, dinv = _host_prep(edge_index)
    nc = _build_nc(meta)

    # fold dinv[src] into x rows (h'[v] = dinv[v] * (x[v] @ W))
    xs = x * dinv[:, None].astype(np.float32)
    xTp = np.zeros((F, NPAD), dtype=bf16)
    xt = np.ascontiguousarray(xs.T).astype(bf16)
    for c in range(NCORES):
        xTp[:, c * SPAD:c * SPAD + SHARD] = xt[:, c * SHARD:(c + 1) * SHARD]
    common = {
        "xT": xTp,
        "wg": np.asarray(W_gcn, np.float32).astype(bf16),
        "w1": np.asarray(W1, np.float32).astype(bf16),
        "w2": np.asarray(W2, np.float32).astype(bf16),
        "w3": np.asarray(W3, np.float32).astype(bf16),
        "bg": np.asarray(b_gcn, np.float32).reshape(64, 1),
        "b1": np.asarray(b1, np.float32).reshape(32, 1),
        "b2": np.asarray(b2, np.float32).reshape(16, 1),
        "b3r": np.asarray(b3, np.float32).reshape(1, 4).astype(bf16),
        "iotam": np.tile(np.arange(128, dtype=np.float32),
                         (128, 1)).astype(bf16),
        "onesr": np.ones((1, 128), dtype=np.float32).astype(bf16),
    }
    in_maps = []
    for c in range(NCORES):
        m = dict(common)
        m["idx"] = idx_streams[c]
        m["dloc"] = dloc_streams[c]
        m["nrm"] = nrm_streams[c]
        in_maps.append(m)

    res = run_bass_kernel_spmd(nc, in_maps, core_ids=list(range(NCORES)),
                               trace=_trace)
    out = np.concatenate(
        [res.results[c]["out"][:SHARD] for c in range(NCORES)], axis=0)
    if _trace:
        kernel.last_exec_time_ns = res.exec_time_ns
    return out


kernel.last_exec_time_ns = None


# revision 10
# speedup vs baseline: 1.9007x; 1.0231x over previous
"""GCN (GCNConv + 3-layer MLP + log_softmax) on 8 Trainium2 NeuronCores.

Strategy (pull-mode message passing):
  - Nodes are sharded 8 ways by destination; each core owns 12500 dst nodes
    (padded to 12544 = 98 tiles of 128).
  - Every core computes the full transformed feature table
    h = (dinv * x) @ W_gcn ([100352, 64] bf16, rows padded, dinv[src]
    folded into x on the host) into its own DRAM — replicating this small
    matmul is cheaper than an AllGather of h.
  - Edges (incl. self-loops) are partitioned by dst shard on the host,
    sorted by (dst tile, src group, src), padded to 128-edge chunks.
    Groups = (table half, row parity): the bf16 table is gathered in
    256-byte units covering a PAIR of rows, so a group fixes which half
    of the gathered pair is the live row.
  - Per chunk the core gathers h row-pairs with dma_gather (256 B/row
    pair); gathers round-robin across all 4 SWDGE queues so descriptor
    generation uses all 8 Q7 cores (4x the single-queue rate).
  - A scaled one-hot S[e, j] = dinv[dst_e] * (dst_local[e] == j) (bf16)
    is built with one fused tensor_scalar op, and the tensor engine
    accumulates aggT[64, 128] += msgs.T @ S in PSUM.
  - The MLP runs in feature-major (transposed) layout so all biases are
    per-partition activation biases; the last matmul flips back to
    node-major and log_softmax finishes on [128, 4] tiles.
"""

import os
import sys

import numpy as np

sys.path.insert(0, "/opt/trn_rl_repo")

N = 100000
F = 256
H = 64
NCLS = 4
NCORES = 8
SHARD = 12500
SPAD = 12544          # 98 * 128
NT = SPAD // 128      # 98 dst tiles per core
NPAD = SPAD * NCORES  # 100352
NG = 4
GSZ = NPAD // NG      # 25088 row-pairs per group half (< 2**15 for int16)
TSB1 = 8              # phase-1 tiles per superblock; 1024-row blocks align
                      # with the half-table boundary (50176 = 49*1024)
TSB3 = 6              # phase-3 dst tiles per superblock
GCALL = 1024          # idxs per dma_gather call (SWDGE ring cap)


def _host_prep(edge_index):
    """Partition/sort/pad edges; returns per-core device arrays + meta."""
    src = np.asarray(edge_index[0]).astype(np.int64)
    dst = np.asarray(edge_index[1]).astype(np.int64)
    deg = np.bincount(dst, minlength=N).astype(np.float64) + 1.0
    dinv = 1.0 / np.sqrt(deg)

    loop = np.arange(N, dtype=np.int64)
    srcA = np.concatenate([src, loop])
    dstA = np.concatenate([dst, loop])
    core = dstA // SHARD
    dl = dstA - core * SHARD
    tl = dl >> 7
    dloc = (dl & 127).astype(np.float32)
    srcp = (srcA // SHARD) * SPAD + (srcA % SHARD)   # padded global src id
    # h rows are stored partition-major per phase-1 superblock (so the
    # h write DMA is contiguous): node srcp lives at h row perm(srcp).
    blk = TSB1 * 128
    b = srcp // blk
    r = srcp - b * blk
    srcp = b * blk + (r % 128) * TSB1 + r // 128
    # groups: (half-table, row parity) — parity selects which half of the
    # gathered 256-byte row pair is live; it also spreads each core's
    # self-loop band over both groups of its half.
    half = srcp // (2 * GSZ)
    w = srcp - half * (2 * GSZ)
    grp = half * 2 + (w & 1)
    idx16 = (w >> 1).astype(np.int16)     # pair index within the half

    key = ((core * NT + tl) * NG + grp)
    order = np.argsort(key * np.int64(NPAD) + srcp, kind="stable")
    key_s = key[order]
    idx_s = idx16[order]
    dloc_s = dloc[order]

    cnt = np.bincount(key, minlength=NCORES * NT * NG).reshape(NCORES, NT, NG)
    C = ((cnt.max(axis=0) + 127) // 128).astype(np.int64)      # [NT, NG] chunks
    starts = np.zeros(NCORES * NT * NG + 1, dtype=np.int64)
    np.cumsum(cnt.reshape(-1), out=starts[1:])

    # superblock partition of the 98 tiles
    sbs = [list(range(s, min(s + TSB3, NT))) for s in range(0, NT, TSB3)]
    # stream layout: for sb: for g: for t in sb: C[t,g] chunks of 128 edges
    col_of = np.zeros((NT, NG), dtype=np.int64)   # chunk column of (t, g)
    sb_meta = []
    col = 0
    for tiles in sbs:
        colbase = col
        Ls = []
        goffs = []
        for g in range(NG):
            goffs.append(col - colbase)
            for t in tiles:
                col_of[t, g] = col
                col += C[t, g]
            Ls.append(int(128 * sum(C[t, g] for t in tiles)))
        sb_meta.append(dict(tiles=tiles, colbase=int(colbase),
                            totc=int(col - colbase), L=Ls, goff=goffs))
    TOTC = int(col)
    TOT = TOTC * 128

    import ml_dtypes
    bf16 = ml_dtypes.bfloat16
    idx_streams, dloc_streams, dinv_rows = [], [], []
    for c in range(NCORES):
        si = np.zeros(TOT, dtype=np.int16)
        sd = np.full(TOT, -1.0, dtype=np.float32)
        for t in range(NT):
            for g in range(NG):
                k = (c * NT + t) * NG + g
                n = cnt[c, t, g]
                if n == 0:
                    continue
                a = starts[k]
                o = col_of[t, g] * 128
                si[o:o + n] = idx_s[a:a + n]
                sd[o:o + n] = dloc_s[a:a + n]
        idx_streams.append(np.tile(si.reshape(-1, 16).T, (8, 1)))      # [128, TOT/16]
        dloc_streams.append(np.ascontiguousarray(sd.reshape(-1, 128).T))  # [128, TOTC]
        dvr = np.zeros((1, SPAD), dtype=np.float32)
        dvr[0, :SHARD] = dinv[c * SHARD:(c + 1) * SHARD]
        dinv_rows.append(dvr.astype(bf16))
    meta = dict(C=C, sb_meta=sb_meta, TOTC=TOTC, TOT=TOT)
    return idx_streams, dloc_streams, dinv_rows, meta, dinv


def _build_nc(meta):
    import concourse.bacc as bacc
    import concourse.mybir as mybir
    import concourse.tile as tile
    from concourse import library_config

    f32 = mybir.dt.float32
    bf16 = mybir.dt.bfloat16
    i16 = mybir.dt.int16
    AF = mybir.ActivationFunctionType
    ALU = mybir.AluOpType
    TOTC, TOT = meta["TOTC"], meta["TOT"]
    C, sb_meta = meta["C"], meta["sb_meta"]

    nc = bacc.Bacc("TRN2", num_swdge_queues=4)
    xT = nc.dram_tensor("xT", [F, NPAD], bf16, kind="ExternalInput")
    wg = nc.dram_tensor("wg", [F, H], bf16, kind="ExternalInput")
    w1 = nc.dram_tensor("w1", [64, 32], bf16, kind="ExternalInput")
    w2 = nc.dram_tensor("w2", [32, 16], bf16, kind="ExternalInput")
    w3 = nc.dram_tensor("w3", [16, 4], bf16, kind="ExternalInput")
    bg = nc.dram_tensor("bg", [64, 1], f32, kind="ExternalInput")
    b1 = nc.dram_tensor("b1", [32, 1], f32, kind="ExternalInput")
    b2 = nc.dram_tensor("b2", [16, 1], f32, kind="ExternalInput")
    b3r = nc.dram_tensor("b3r", [1, 4], bf16, kind="ExternalInput")
    iotam = nc.dram_tensor("iotam", [128, 128], bf16, kind="ExternalInput")
    onesr = nc.dram_tensor("onesr", [1, 128], bf16, kind="ExternalInput")
    idxT = nc.dram_tensor("idx", [128, TOT // 16], i16, kind="ExternalInput")
    dlocT = nc.dram_tensor("dloc", [128, TOTC], f32, kind="ExternalInput")
    dvT = nc.dram_tensor("dv", [1, SPAD], bf16, kind="ExternalInput")
    outT = nc.dram_tensor("out", [SPAD, NCLS], f32, kind="ExternalOutput")

    NT1 = NPAD // 128  # 784 phase-1 tiles
    sb1 = [list(range(s, min(s + TSB1, NT1))) for s in range(0, NT1, TSB1)]
    # per-pass (groups 0-1 / groups 2-3) chunk-count maxima for tile sizing
    maxc0 = max(m["goff"][2] for m in sb_meta)
    maxc1 = max(m["totc"] - m["goff"][2] for m in sb_meta)
    maxc = max(maxc0, maxc1)

    with tile.TileContext(nc) as tc:
        with tc.tile_pool(name="const", bufs=1) as cp, \
             tc.tile_pool(name="dram", bufs=1, space="DRAM") as dram:
            # each half stored as row pairs: [25088 pairs, 128] bf16
            h01 = dram.tile([GSZ, 2 * H], bf16, tag="h01")
            h23 = dram.tile([GSZ, 2 * H], bf16, tag="h23")
            nc.gpsimd.load_library(library_config.mlp)

            wg0 = cp.tile([128, H], bf16, tag="wg0")
            wg1 = cp.tile([128, H], bf16, tag="wg1")
            nc.sync.dma_start(wg0[:], wg[0:128, :])
            nc.sync.dma_start(wg1[:], wg[128:256, :])
            w1s = cp.tile([64, 32], bf16, tag="w1s")
            w2s = cp.tile([32, 16], bf16, tag="w2s")
            w3s = cp.tile([16, 4], bf16, tag="w3s")
            bgs = cp.tile([64, 1], f32, tag="bgs")
            b1s = cp.tile([32, 1], f32, tag="b1s")
            b2s = cp.tile([16, 1], f32, tag="b2s")
            b3s = cp.tile([1, 4], bf16, tag="b3s")
            iots = cp.tile([128, 128], bf16, tag="iots")
            ones = cp.tile([1, 128], bf16, tag="ones")
            dvs = cp.tile([1, SPAD], bf16, tag="dvs")
            nc.sync.dma_start(dvs[:], dvT[:, :])
            for t_, d_ in ((w1s, w1), (w2s, w2), (w3s, w3), (bgs, bg),
                           (b1s, b1), (b2s, b2), (b3s, b3r), (iots, iotam),
                           (ones, onesr)):
                nc.sync.dma_start(t_[:], d_[:, :])

            # All pools stay open so phase 1 overlaps the pass-0 gathers
            # (closing/reusing SBUF zones would add false dependencies).
            with tc.tile_pool(name="p1", bufs=2) as p1p, \
                 tc.tile_pool(name="ps1", bufs=2, space="PSUM") as ps1, \
                 tc.tile_pool(name="p3", bufs=3) as p3p, \
                 tc.tile_pool(name="gb", bufs=3) as gbp, \
                 tc.tile_pool(name="sp", bufs=6) as sp, \
                 tc.tile_pool(name="ep", bufs=3) as ep, \
                 tc.tile_pool(name="oa", bufs=1) as oap, \
                 tc.tile_pool(name="agg", bufs=3, space="PSUM") as aggp, \
                 tc.tile_pool(name="ddp", bufs=1, space="PSUM") as ddp, \
                 tc.tile_pool(name="mlp", bufs=2, space="PSUM") as mlpp:
                # -------- phase 1: h = x @ W_gcn, halves written in order ---
                nhalf = len(sb1) // 2
                for bi, tiles in enumerate(sb1):
                    T = len(tiles)
                    t0 = tiles[0]
                    xt0 = p1p.tile([128, TSB1 * 128], bf16, tag="xt0")
                    xt1 = p1p.tile([128, TSB1 * 128], bf16, tag="xt1")
                    nc.sync.dma_start(
                        xt0[:, :T * 128], xT[0:128, t0 * 128:(t0 + T) * 128])
                    nc.sync.dma_start(
                        xt1[:, :T * 128], xT[128:256, t0 * 128:(t0 + T) * 128])
                    hsb = p1p.tile([128, TSB1 * H], bf16, tag="hsb")
                    for i in range(T):
                        ps = ps1.tile([128, H], f32, tag="hps")
                        nc.tensor.matmul(ps[:], xt0[:, i * 128:(i + 1) * 128],
                                         wg0[:], start=True, stop=False)
                        nc.tensor.matmul(ps[:], xt1[:, i * 128:(i + 1) * 128],
                                         wg1[:], start=False, stop=True)
                        nc.scalar.activation(hsb[:, i * H:(i + 1) * H], ps[:],
                                             AF.Copy)
                    hP = h01 if bi < nhalf else h23
                    r0 = (bi if bi < nhalf else bi - nhalf) * TSB1 * 128
                    # partition-major row order -> per-partition contiguous
                    # 1 KB runs: partition p holds rows r0+p*T..r0+p*T+T-1,
                    # i.e. pair rows (r0//2)+p*(T//2).. of the [GSZ,128] table
                    nc.sync.dma_start(
                        hP[r0 // 2:(r0 + T * 128) // 2, :]
                        .rearrange("(p q) f -> p q f", p=128),
                        hsb[:].rearrange("p (q f) -> p q f", q=T // 2))

                # -------- phase 3: two passes (half 0, then half 1) --------
                outacc = oap.tile([128, NT, NCLS], f32, tag="outacc")
                accT = oap.tile([64, NT * 128], f32, tag="accT")
                gq = 0
                for pas in (0, 1):
                    hP = h01 if pas == 0 else h23
                    gl, gh = 2 * pas, 2 * pas + 2
                    for m in sb_meta:
                        tiles = m["tiles"]
                        pco = m["goff"][gl]                  # pass col offset
                        pend = m["totc"] if pas else m["goff"][2]
                        ptc = pend - pco                     # pass chunk count
                        cb = m["colbase"] + pco              # global col base
                        idxsb = p3p.tile([128, maxc * 8], i16, tag="idx")
                        nc.sync.dma_start(idxsb[:, :ptc * 8],
                                          idxT[:, cb * 8:(cb + ptc) * 8])
                        dlsb = p3p.tile([128, maxc], f32, tag="dl")
                        nc.sync.dma_start(dlsb[:, :ptc],
                                          dlocT[:, cb:cb + ptc])
                        gbuf = gbp.tile([128, maxc, 2 * H], bf16, tag="gbuf")
                        for g in range(gl, gh):
                            L = m["L"][g]
                            go = m["goff"][g] - pco
                            for k in range(0, L, GCALL):
                                ni = min(GCALL, L - k)
                                c0 = go + k // 128
                                nc.gpsimd.dma_gather(
                                    gbuf[:, c0:c0 + ni // 128, :],
                                    hP[:],
                                    idxsb[:, c0 * 8:(c0 + ni // 128) * 8],
                                    ni, ni, 2 * H, queue_num=gq % 4)
                                gq += 1
                        for ti, t in enumerate(tiles):
                            agg = aggp.tile([64, 128], f32, tag="agg")
                            nch = int(C[t, gl:gh].sum())
                            done = 0
                            for g in range(gl, gh):
                                base = (m["goff"][g] - pco) + int(
                                    sum(C[tt, g] for tt in tiles[:ti]))
                                par = g & 1
                                for j in range(int(C[t, g])):
                                    pos = base + j
                                    S = sp.tile([128, 128], bf16, tag="S")
                                    nc.vector.tensor_scalar(
                                        S[:], iots[:], dlsb[:, pos:pos + 1],
                                        None, op0=ALU.is_equal)
                                    nc.tensor.matmul(
                                        agg[:],
                                        gbuf[:, pos, par * H:(par + 1) * H],
                                        S[:],
                                        start=(done == 0),
                                        stop=(done == nch - 1))
                                    done += 1
                            if pas == 0:
                                if nch == 0:
                                    nc.vector.memset(
                                        accT[:, t * 128:(t + 1) * 128], 0.0)
                                else:
                                    nc.vector.tensor_copy(
                                        accT[:, t * 128:(t + 1) * 128], agg[:])
                                continue
                            dd = ddp.tile([64, 128], f32, tag="dd")
                            nc.tensor.matmul(dd[:], ones[0:1, :64],
                                             dvs[0:1, t * 128:(t + 1) * 128],
                                             start=True, stop=True)
                            t0p = ep.tile([64, 128], f32, tag="t0p")
                            nc.vector.tensor_add(
                                t0p[:], accT[:, t * 128:(t + 1) * 128], agg[:])
                            t0q = ep.tile([64, 128], f32, tag="t0q")
                            nc.vector.tensor_mul(t0q[:], t0p[:], dd[:])
                            t0s = ep.tile([64, 128], bf16, tag="t0")
                            nc.scalar.activation(t0s[:], t0q[:], AF.Relu,
                                                 bias=bgs[:])
                            pm1 = mlpp.tile([32, 128], f32, tag="pm")
                            nc.tensor.matmul(pm1[:], w1s[:], t0s[:],
                                             start=True, stop=True)
                            t1s = ep.tile([32, 128], bf16, tag="t1")
                            nc.scalar.activation(t1s[:], pm1[:], AF.Relu,
                                                 bias=b1s[:])
                            pm2 = mlpp.tile([16, 128], f32, tag="pm")
                            nc.tensor.matmul(pm2[:], w2s[:], t1s[:],
                                             start=True, stop=True)
                            t2s = ep.tile([16, 128], bf16, tag="t2")
                            nc.scalar.activation(t2s[:], pm2[:], AF.Relu,
                                                 bias=b2s[:])
                            pm3 = mlpp.tile([128, NCLS], f32, tag="pm")
                            nc.tensor.matmul(pm3[:], t2s[:], w3s[:],
                                             start=True, stop=False)
                            nc.tensor.matmul(pm3[:], ones[:], b3s[:],
                                             start=False, stop=True)
                            nmax = ep.tile([128, 1], f32, tag="nmax")
                            nc.vector.tensor_reduce(nmax[:], pm3[:],
                                                    axis=mybir.AxisListType.X,
                                                    op=ALU.max, negate=True)
                            esb = ep.tile([128, NCLS], f32, tag="esb")
                            ssum = ep.tile([128, 1], f32, tag="ssum")
                            nc.scalar.activation(esb[:], pm3[:], AF.Exp,
                                                 bias=nmax[:], accum_out=ssum[:])
                            lsb = ep.tile([128, 1], f32, tag="lsb")
                            nc.scalar.activation(lsb[:], ssum[:], AF.Ln)
                            nc.vector.tensor_scalar(
                                outacc[:, t, :], pm3[:], nmax[:], lsb[:],
                                op0=ALU.add, op1=ALU.subtract)
                nc.sync.dma_start(
                    outT[:, :].rearrange("(t p) c -> p t c", p=128),
                    outacc[:])
    nc.compile()
    return nc


def kernel(x, edge_index, W_gcn, b_gcn, W1, b1, W2, b2, W3, b3,
           _trace=False):
    import ml_dtypes
    from concourse.bass_utils import run_bass_kernel_spmd

    bf16 = ml_dtypes.bfloat16
    x = np.asarray(x, dtype=np.float32)
    idx_streams, dloc_streams, dinv_rows, meta, dinv = _host_prep(edge_index)
    nc = _build_nc(meta)

    # fold dinv[src] into x rows (h'[v] = dinv[v] * (x[v] @ W))
    xs = x * dinv[:, None].astype(np.float32)
    xTp = np.zeros((F, NPAD), dtype=bf16)
    xt = np.ascontiguousarray(xs.T).astype(bf16)
    for c in range(NCORES):
        xTp[:, c * SPAD:c * SPAD + SHARD] = xt[:, c * SHARD:(c + 1) * SHARD]
    common = {
        "xT": xTp,
        "wg": np.asarray(W_gcn, np.float32).astype(bf16),
        "w1": np.asarray(W1, np.float32).astype(bf16),
        "w2": np.asarray(W2, np.float32).astype(bf16),
        "w3": np.asarray(W3, np.float32).astype(bf16),
        "bg": np.asarray(b_gcn, np.float32).reshape(64, 1),
        "b1": np.asarray(b1, np.float32).reshape(32, 1),
        "b2": np.asarray(b2, np.float32).reshape(16, 1),
        "b3r": np.asarray(b3, np.float32).reshape(1, 4).astype(bf16),
        "iotam": np.tile(np.arange(128, dtype=np.float32),
                         (128, 1)).astype(bf16),
        "onesr": np.ones((1, 128), dtype=np.float32).astype(bf16),
    }
    in_maps = []
    for c in range(NCORES):
        m = dict(common)
        m["idx"] = idx_streams[c]
        m["dloc"] = dloc_streams[c]
        m["dv"] = dinv_rows[c]
        in_maps.append(m)

    res = run_bass_kernel_spmd(nc, in_maps, core_ids=list(range(NCORES)),
                               trace=_trace)
    out = np.concatenate(
        [res.results[c]["out"][:SHARD] for c in range(NCORES)], axis=0)
    if _trace:
        kernel.last_exec_time_ns = res.exec_time_ns
    return out


kernel.last_exec_time_ns = None


# revision 11
# speedup vs baseline: 1.9509x; 1.0264x over previous
"""GCN (GCNConv + 3-layer MLP + log_softmax) on 8 Trainium2 NeuronCores.

Strategy (pull-mode message passing):
  - Nodes are sharded 8 ways by destination; each core owns 12500 dst nodes
    (padded to 12544 = 98 tiles of 128).
  - Every core computes the full transformed feature table
    h = (dinv * x) @ W_gcn ([100352, 64] bf16, rows padded, dinv[src]
    folded into x on the host) into its own DRAM — replicating this small
    matmul is cheaper than an AllGather of h.
  - Edges (incl. self-loops) are partitioned by dst shard on the host,
    sorted by (dst tile, src group, src), padded to 128-edge chunks.
    Groups = (table half, row parity): the bf16 table is gathered in
    256-byte units covering a PAIR of rows, so a group fixes which half
    of the gathered pair is the live row.
  - Per chunk the core gathers h row-pairs with dma_gather (256 B/row
    pair); gathers round-robin across all 4 SWDGE queues so descriptor
    generation uses all 8 Q7 cores (4x the single-queue rate).
  - A scaled one-hot S[e, j] = dinv[dst_e] * (dst_local[e] == j) (bf16)
    is built with one fused tensor_scalar op, and the tensor engine
    accumulates aggT[64, 128] += msgs.T @ S in PSUM.
  - The MLP runs in feature-major (transposed) layout so all biases are
    per-partition activation biases; the last matmul flips back to
    node-major and log_softmax finishes on [128, 4] tiles.
"""

import os
import sys

import numpy as np

sys.path.insert(0, "/opt/trn_rl_repo")

N = 100000
F = 256
H = 64
NCLS = 4
NCORES = 8
SHARD = 12500
SPAD = 12544          # 98 * 128
NT = SPAD // 128      # 98 dst tiles per core
NPAD = SPAD * NCORES  # 100352
NG = 4
GSZ = NPAD // NG      # 25088 row-pairs per group half (< 2**15 for int16)
TSB1 = 8              # phase-1 tiles per superblock; 1024-row blocks align
                      # with the half-table boundary (50176 = 49*1024)
TSB3 = 6              # phase-3 dst tiles per superblock
GCALL = 1024          # idxs per dma_gather call (SWDGE ring cap)
KS = 32               # S one-hot chunks built per DVE instruction


def _host_prep(edge_index):
    """Partition/sort/pad edges; returns per-core device arrays + meta."""
    src = np.asarray(edge_index[0]).astype(np.int64)
    dst = np.asarray(edge_index[1]).astype(np.int64)
    deg = np.bincount(dst, minlength=N).astype(np.float64) + 1.0
    dinv = 1.0 / np.sqrt(deg)

    loop = np.arange(N, dtype=np.int64)
    srcA = np.concatenate([src, loop])
    dstA = np.concatenate([dst, loop])
    core = dstA // SHARD
    dl = dstA - core * SHARD
    tl = dl >> 7
    dloc = (dl & 127).astype(np.float32)
    srcp = (srcA // SHARD) * SPAD + (srcA % SHARD)   # padded global src id
    # h rows are stored partition-major per phase-1 superblock (so the
    # h write DMA is contiguous): node srcp lives at h row perm(srcp).
    blk = TSB1 * 128
    b = srcp // blk
    r = srcp - b * blk
    srcp = b * blk + (r % 128) * TSB1 + r // 128
    # groups: (half-table, row parity) — parity selects which half of the
    # gathered 256-byte row pair is live; it also spreads each core's
    # self-loop band over both groups of its half.
    half = srcp // (2 * GSZ)
    w = srcp - half * (2 * GSZ)
    grp = half * 2 + (w & 1)
    idx16 = (w >> 1).astype(np.int16)     # pair index within the half

    key = ((core * NT + tl) * NG + grp)
    order = np.argsort(key * np.int64(NPAD) + srcp, kind="stable")
    key_s = key[order]
    idx_s = idx16[order]
    dloc_s = dloc[order]

    cnt = np.bincount(key, minlength=NCORES * NT * NG).reshape(NCORES, NT, NG)
    C = ((cnt.max(axis=0) + 127) // 128).astype(np.int64)      # [NT, NG] chunks
    starts = np.zeros(NCORES * NT * NG + 1, dtype=np.int64)
    np.cumsum(cnt.reshape(-1), out=starts[1:])

    # superblock partition of the 98 tiles
    sbs = [list(range(s, min(s + TSB3, NT))) for s in range(0, NT, TSB3)]
    # stream layout: for sb: for g: for t in sb: C[t,g] chunks of 128 edges
    col_of = np.zeros((NT, NG), dtype=np.int64)   # chunk column of (t, g)
    sb_meta = []
    col = 0
    for tiles in sbs:
        colbase = col
        Ls = []
        goffs = []
        for g in range(NG):
            goffs.append(col - colbase)
            for t in tiles:
                col_of[t, g] = col
                col += C[t, g]
            Ls.append(int(128 * sum(C[t, g] for t in tiles)))
        sb_meta.append(dict(tiles=tiles, colbase=int(colbase),
                            totc=int(col - colbase), L=Ls, goff=goffs))
    TOTC = int(col)
    TOT = TOTC * 128

    import ml_dtypes
    bf16 = ml_dtypes.bfloat16
    idx_streams, dloc_streams, dinv_rows = [], [], []
    for c in range(NCORES):
        si = np.zeros(TOT, dtype=np.int16)
        sd = np.full(TOT, -1.0, dtype=np.float32)
        for t in range(NT):
            for g in range(NG):
                k = (c * NT + t) * NG + g
                n = cnt[c, t, g]
                if n == 0:
                    continue
                a = starts[k]
                o = col_of[t, g] * 128
                si[o:o + n] = idx_s[a:a + n]
                sd[o:o + n] = dloc_s[a:a + n]
        idx_streams.append(np.tile(si.reshape(-1, 16).T, (8, 1)))      # [128, TOT/16]
        dloc_streams.append(
            np.ascontiguousarray(sd.reshape(-1, 128).T).astype(bf16))  # [128, TOTC]
        dvr = np.zeros((1, SPAD), dtype=np.float32)
        dvr[0, :SHARD] = dinv[c * SHARD:(c + 1) * SHARD]
        dinv_rows.append(dvr.astype(bf16))
    meta = dict(C=C, sb_meta=sb_meta, TOTC=TOTC, TOT=TOT)
    return idx_streams, dloc_streams, dinv_rows, meta, dinv


def _build_nc(meta):
    import concourse.bacc as bacc
    import concourse.mybir as mybir
    import concourse.tile as tile
    from concourse import library_config

    f32 = mybir.dt.float32
    bf16 = mybir.dt.bfloat16
    i16 = mybir.dt.int16
    AF = mybir.ActivationFunctionType
    ALU = mybir.AluOpType
    TOTC, TOT = meta["TOTC"], meta["TOT"]
    C, sb_meta = meta["C"], meta["sb_meta"]

    nc = bacc.Bacc("TRN2", num_swdge_queues=4)
    xT = nc.dram_tensor("xT", [F, NPAD], bf16, kind="ExternalInput")
    wg = nc.dram_tensor("wg", [F, H], bf16, kind="ExternalInput")
    w1 = nc.dram_tensor("w1", [64, 32], bf16, kind="ExternalInput")
    w2 = nc.dram_tensor("w2", [32, 16], bf16, kind="ExternalInput")
    w3 = nc.dram_tensor("w3", [16, 4], bf16, kind="ExternalInput")
    bg = nc.dram_tensor("bg", [64, 1], f32, kind="ExternalInput")
    b1 = nc.dram_tensor("b1", [32, 1], f32, kind="ExternalInput")
    b2 = nc.dram_tensor("b2", [16, 1], f32, kind="ExternalInput")
    b3r = nc.dram_tensor("b3r", [1, 4], bf16, kind="ExternalInput")
    iotam = nc.dram_tensor("iotam", [128, 128], bf16, kind="ExternalInput")
    onesr = nc.dram_tensor("onesr", [1, 128], bf16, kind="ExternalInput")
    idxT = nc.dram_tensor("idx", [128, TOT // 16], i16, kind="ExternalInput")
    dlocT = nc.dram_tensor("dloc", [128, TOTC], bf16, kind="ExternalInput")
    dvT = nc.dram_tensor("dv", [1, SPAD], bf16, kind="ExternalInput")
    outT = nc.dram_tensor("out", [SPAD, NCLS], f32, kind="ExternalOutput")

    NT1 = NPAD // 128  # 784 phase-1 tiles
    sb1 = [list(range(s, min(s + TSB1, NT1))) for s in range(0, NT1, TSB1)]
    # per-pass (groups 0-1 / groups 2-3) chunk-count maxima for tile sizing
    maxc0 = max(m["goff"][2] for m in sb_meta)
    maxc1 = max(m["totc"] - m["goff"][2] for m in sb_meta)
    maxc = max(maxc0, maxc1)

    with tile.TileContext(nc) as tc:
        with tc.tile_pool(name="const", bufs=1) as cp, \
             tc.tile_pool(name="dram", bufs=1, space="DRAM") as dram:
            # each half stored as row pairs: [25088 pairs, 128] bf16
            h01 = dram.tile([GSZ, 2 * H], bf16, tag="h01")
            h23 = dram.tile([GSZ, 2 * H], bf16, tag="h23")
            nc.gpsimd.load_library(library_config.mlp)

            wg0 = cp.tile([128, H], bf16, tag="wg0")
            wg1 = cp.tile([128, H], bf16, tag="wg1")
            nc.sync.dma_start(wg0[:], wg[0:128, :])
            nc.sync.dma_start(wg1[:], wg[128:256, :])
            w1s = cp.tile([64, 32], bf16, tag="w1s")
            w2s = cp.tile([32, 16], bf16, tag="w2s")
            w3s = cp.tile([16, 4], bf16, tag="w3s")
            bgs = cp.tile([64, 1], f32, tag="bgs")
            b1s = cp.tile([32, 1], f32, tag="b1s")
            b2s = cp.tile([16, 1], f32, tag="b2s")
            b3s = cp.tile([1, 4], bf16, tag="b3s")
            iots = cp.tile([128, 128], bf16, tag="iots")
            ones = cp.tile([1, 128], bf16, tag="ones")
            dvs = cp.tile([1, SPAD], bf16, tag="dvs")
            nc.sync.dma_start(dvs[:], dvT[:, :])
            for t_, d_ in ((w1s, w1), (w2s, w2), (w3s, w3), (bgs, bg),
                           (b1s, b1), (b2s, b2), (b3s, b3r), (iots, iotam),
                           (ones, onesr)):
                nc.sync.dma_start(t_[:], d_[:, :])

            # All pools stay open so phase 1 overlaps the pass-0 gathers
            # (closing/reusing SBUF zones would add false dependencies).
            with tc.tile_pool(name="p1", bufs=2) as p1p, \
                 tc.tile_pool(name="ps1", bufs=2, space="PSUM") as ps1, \
                 tc.tile_pool(name="p3", bufs=3) as p3p, \
                 tc.tile_pool(name="gb", bufs=2) as gbp, \
                 tc.tile_pool(name="sp", bufs=6) as sp, \
                 tc.tile_pool(name="ep", bufs=3) as ep, \
                 tc.tile_pool(name="oa", bufs=1) as oap, \
                 tc.tile_pool(name="agg", bufs=3, space="PSUM") as aggp, \
                 tc.tile_pool(name="ddp", bufs=1, space="PSUM") as ddp, \
                 tc.tile_pool(name="mlp", bufs=2, space="PSUM") as mlpp:
                # -------- phase 1: h = x @ W_gcn, halves written in order ---
                nhalf = len(sb1) // 2
                for bi, tiles in enumerate(sb1):
                    T = len(tiles)
                    t0 = tiles[0]
                    xt0 = p1p.tile([128, TSB1 * 128], bf16, tag="xt0")
                    xt1 = p1p.tile([128, TSB1 * 128], bf16, tag="xt1")
                    nc.sync.dma_start(
                        xt0[:, :T * 128], xT[0:128, t0 * 128:(t0 + T) * 128])
                    nc.sync.dma_start(
                        xt1[:, :T * 128], xT[128:256, t0 * 128:(t0 + T) * 128])
                    hsb = p1p.tile([128, TSB1 * H], bf16, tag="hsb")
                    for i in range(T):
                        ps = ps1.tile([128, H], f32, tag="hps")
                        nc.tensor.matmul(ps[:], xt0[:, i * 128:(i + 1) * 128],
                                         wg0[:], start=True, stop=False)
                        nc.tensor.matmul(ps[:], xt1[:, i * 128:(i + 1) * 128],
                                         wg1[:], start=False, stop=True)
                        nc.scalar.activation(hsb[:, i * H:(i + 1) * H], ps[:],
                                             AF.Copy)
                    hP = h01 if bi < nhalf else h23
                    r0 = (bi if bi < nhalf else bi - nhalf) * TSB1 * 128
                    # partition-major row order -> per-partition contiguous
                    # 1 KB runs: partition p holds rows r0+p*T..r0+p*T+T-1,
                    # i.e. pair rows (r0//2)+p*(T//2).. of the [GSZ,128] table
                    nc.sync.dma_start(
                        hP[r0 // 2:(r0 + T * 128) // 2, :]
                        .rearrange("(p q) f -> p q f", p=128),
                        hsb[:].rearrange("p (q f) -> p q f", q=T // 2))

                # -------- phase 3: two passes (half 0, then half 1) --------
                outacc = oap.tile([128, NT, NCLS], f32, tag="outacc")
                accT = oap.tile([64, NT * 128], f32, tag="accT")
                gq = 0
                for pas in (0, 1):
                    hP = h01 if pas == 0 else h23
                    gl, gh = 2 * pas, 2 * pas + 2
                    for m in sb_meta:
                        tiles = m["tiles"]
                        pco = m["goff"][gl]                  # pass col offset
                        pend = m["totc"] if pas else m["goff"][2]
                        ptc = pend - pco                     # pass chunk count
                        cb = m["colbase"] + pco              # global col base
                        idxsb = p3p.tile([128, maxc * 8], i16, tag="idx")
                        nc.sync.dma_start(idxsb[:, :ptc * 8],
                                          idxT[:, cb * 8:(cb + ptc) * 8])
                        dlsb = p3p.tile([128, maxc], bf16, tag="dl")
                        nc.sync.dma_start(dlsb[:, :ptc],
                                          dlocT[:, cb:cb + ptc])
                        gbuf = gbp.tile([128, maxc, 2 * H], bf16, tag="gbuf")
                        slist = []
                        for b0 in range(0, ptc, KS):
                            kk = min(KS, ptc - b0)
                            St = sp.tile([128, KS, 128], bf16, tag="S")
                            nc.vector.scalar_tensor_tensor(
                                St[:, :kk, :],
                                iots[:].unsqueeze(1)
                                    .broadcast_to([128, kk, 128]),
                                0.0,
                                dlsb[:, b0:b0 + kk].unsqueeze(2)
                                    .broadcast_to([128, kk, 128]),
                                op0=ALU.subtract, op1=ALU.is_equal)
                            slist.append(St)
                        for g in range(gl, gh):
                            L = m["L"][g]
                            go = m["goff"][g] - pco
                            for k in range(0, L, GCALL):
                                ni = min(GCALL, L - k)
                                c0 = go + k // 128
                                nc.gpsimd.dma_gather(
                                    gbuf[:, c0:c0 + ni // 128, :],
                                    hP[:],
                                    idxsb[:, c0 * 8:(c0 + ni // 128) * 8],
                                    ni, ni, 2 * H, queue_num=gq % 4)
                                gq += 1
                        for ti, t in enumerate(tiles):
                            agg = aggp.tile([64, 128], f32, tag="agg")
                            nch = int(C[t, gl:gh].sum())
                            done = 0
                            for g in range(gl, gh):
                                base = (m["goff"][g] - pco) + int(
                                    sum(C[tt, g] for tt in tiles[:ti]))
                                par = g & 1
                                for j in range(int(C[t, g])):
                                    pos = base + j
                                    nc.tensor.matmul(
                                        agg[:],
                                        gbuf[:, pos, par * H:(par + 1) * H],
                                        slist[pos // KS][:, pos % KS, :],
                                        start=(done == 0),
                                        stop=(done == nch - 1))
                                    done += 1
                            if pas == 0:
                                if nch == 0:
                                    nc.vector.memset(
                                        accT[:, t * 128:(t + 1) * 128], 0.0)
                                else:
                                    nc.vector.tensor_copy(
                                        accT[:, t * 128:(t + 1) * 128], agg[:])
                                continue
                            dd = ddp.tile([64, 128], f32, tag="dd")
                            nc.tensor.matmul(dd[:], ones[0:1, :64],
                                             dvs[0:1, t * 128:(t + 1) * 128],
                                             start=True, stop=True)
                            t0p = ep.tile([64, 128], f32, tag="t0p")
                            nc.vector.tensor_add(
                                t0p[:], accT[:, t * 128:(t + 1) * 128], agg[:])
                            t0q = ep.tile([64, 128], f32, tag="t0q")
                            nc.vector.tensor_mul(t0q[:], t0p[:], dd[:])
                            t0s = ep.tile([64, 128], bf16, tag="t0")
                            nc.scalar.activation(t0s[:], t0q[:], AF.Relu,
                                                 bias=bgs[:])
                            pm1 = mlpp.tile([32, 128], f32, tag="pm")
                            nc.tensor.matmul(pm1[:], w1s[:], t0s[:],
                                             start=True, stop=True)
                            t1s = ep.tile([32, 128], bf16, tag="t1")
                            nc.scalar.activation(t1s[:], pm1[:], AF.Relu,
                                                 bias=b1s[:])
                            pm2 = mlpp.tile([16, 128], f32, tag="pm")
                            nc.tensor.matmul(pm2[:], w2s[:], t1s[:],
                                             start=True, stop=True)
                            t2s = ep.tile([16, 128], bf16, tag="t2")
                            nc.scalar.activation(t2s[:], pm2[:], AF.Relu,
                                                 bias=b2s[:])
                            pm3 = mlpp.tile([128, NCLS], f32, tag="pm")
                            nc.tensor.matmul(pm3[:], t2s[:], w3s[:],
                                             start=True, stop=False)
                            nc.tensor.matmul(pm3[:], ones[:], b3s[:],
                                             start=False, stop=True)
                            nmax = ep.tile([128, 1], f32, tag="nmax")
                            nc.vector.tensor_reduce(nmax[:], pm3[:],
                                                    axis=mybir.AxisListType.X,
                                                    op=ALU.max, negate=True)
                            esb = ep.tile([128, NCLS], f32, tag="esb")
                            ssum = ep.tile([128, 1], f32, tag="ssum")
                            nc.scalar.activation(esb[:], pm3[:], AF.Exp,
                                                 bias=nmax[:], accum_out=ssum[:])
                            lsb = ep.tile([128, 1], f32, tag="lsb")
                            nc.scalar.activation(lsb[:], ssum[:], AF.Ln)
                            nc.vector.tensor_scalar(
                                outacc[:, t, :], pm3[:], nmax[:], lsb[:],
                                op0=ALU.add, op1=ALU.subtract)
                nc.sync.dma_start(
                    outT[:, :].rearrange("(t p) c -> p t c", p=128),
                    outacc[:])
    nc.compile()
    return nc


def kernel(x, edge_index, W_gcn, b_gcn, W1, b1, W2, b2, W3, b3,
           _trace=False):
    import ml_dtypes
    from concourse.bass_utils import run_bass_kernel_spmd

    bf16 = ml_dtypes.bfloat16
    x = np.asarray(x, dtype=np.float32)
    idx_streams, dloc_streams, dinv_rows, meta, dinv = _host_prep(edge_index)
    nc = _build_nc(meta)

    # fold dinv[src] into x rows (h'[v] = dinv[v] * (x[v] @ W))
    xs = x * dinv[:, None].astype(np.float32)
    xTp = np.zeros((F, NPAD), dtype=bf16)
    xt = np.ascontiguousarray(xs.T).astype(bf16)
    for c in range(NCORES):
        xTp[:, c * SPAD:c * SPAD + SHARD] = xt[:, c * SHARD:(c + 1) * SHARD]
    common = {
        "xT": xTp,
        "wg": np.asarray(W_gcn, np.float32).astype(bf16),
        "w1": np.asarray(W1, np.float32).astype(bf16),
        "w2": np.asarray(W2, np.float32).astype(bf16),
        "w3": np.asarray(W3, np.float32).astype(bf16),
        "bg": np.asarray(b_gcn, np.float32).reshape(64, 1),
        "b1": np.asarray(b1, np.float32).reshape(32, 1),
        "b2": np.asarray(b2, np.float32).reshape(16, 1),
        "b3r": np.asarray(b3, np.float32).reshape(1, 4).astype(bf16),
        "iotam": np.tile(np.arange(128, dtype=np.float32),
                         (128, 1)).astype(bf16),
        "onesr": np.ones((1, 128), dtype=np.float32).astype(bf16),
    }
    in_maps = []
    for c in range(NCORES):
        m = dict(common)
        m["idx"] = idx_streams[c]
        m["dloc"] = dloc_streams[c]
        m["dv"] = dinv_rows[c]
        in_maps.append(m)

    res = run_bass_kernel_spmd(nc, in_maps, core_ids=list(range(NCORES)),
                               trace=_trace)
    out = np.concatenate(
        [res.results[c]["out"][:SHARD] for c in range(NCORES)], axis=0)
    if _trace:
        kernel.last_exec_time_ns = res.exec_time_ns
    return out


kernel.last_exec_time_ns = None


# revision 14
# speedup vs baseline: 2.6439x; 1.3552x over previous
"""GCN (GCNConv + 3-layer MLP + log_softmax) on 8 Trainium2 NeuronCores.

Strategy (pull-mode message passing):
  - Nodes are sharded 8 ways by destination; each core owns 12500 dst nodes
    (padded to 12544 = 98 tiles of 128).
  - Every core computes the full transformed feature table
    h = (dinv * x) @ W_gcn ([100352, 64] bf16, rows padded, dinv[src]
    folded into x on the host) into its own DRAM — replicating this small
    matmul is cheaper than an AllGather of h.
  - Edges (incl. self-loops) are partitioned by dst shard on the host,
    sorted by (dst tile, src group, src), padded to 128-edge chunks.
    Groups = (table half, row parity): the bf16 table is gathered in
    256-byte units covering a PAIR of rows, so a group fixes which half
    of the gathered pair is the live row.
  - Per chunk the core gathers h row-pairs with dma_gather (256 B/row
    pair); gathers round-robin across all 4 SWDGE queues so descriptor
    generation uses all 8 Q7 cores (4x the single-queue rate).
  - A scaled one-hot S[e, j] = dinv[dst_e] * (dst_local[e] == j) (bf16)
    is built with one fused tensor_scalar op, and the tensor engine
    accumulates aggT[64, 128] += msgs.T @ S in PSUM.
  - The MLP runs in feature-major (transposed) layout so all biases are
    per-partition activation biases; the last matmul flips back to
    node-major and log_softmax finishes on [128, 4] tiles.
"""

import os
import sys

import numpy as np

sys.path.insert(0, "/opt/trn_rl_repo")

N = 100000
F = 256
H = 64
NCLS = 4
NCORES = 8
SHARD = 12500
SPAD = 12544          # 98 * 128
NT = SPAD // 128      # 98 dst tiles per core
NPAD = SPAD * NCORES  # 100352
NG = 4
GSZ = NPAD // NG      # 25088 row-pairs per group half (< 2**15 for int16)
TSB1 = 8              # phase-1 tiles per superblock; 1024-row blocks align
                      # with the half-table boundary (50176 = 49*1024)
TSB3 = 6              # phase-3 dst tiles per superblock
GCALL = 1024          # idxs per dma_gather call (SWDGE ring cap)
KS = 32               # S one-hot chunks built per DVE instruction


def _host_prep(edge_index):
    """Partition/sort/pad edges; returns per-core device arrays + meta."""
    src = np.asarray(edge_index[0]).astype(np.int64)
    dst = np.asarray(edge_index[1]).astype(np.int64)
    deg = np.bincount(dst, minlength=N).astype(np.float64) + 1.0
    dinv = 1.0 / np.sqrt(deg)

    loop = np.arange(N, dtype=np.int64)
    srcA = np.concatenate([src, loop])
    dstA = np.concatenate([dst, loop])
    core = dstA // SHARD
    dl = dstA - core * SHARD
    tl = dl >> 7
    dloc = (dl & 127).astype(np.float32)
    srcp = (srcA // SHARD) * SPAD + (srcA % SHARD)   # padded global src id
    # h rows are stored partition-major per phase-1 superblock (so the
    # h write DMA is contiguous): node srcp lives at h row perm(srcp).
    blk = TSB1 * 128
    b = srcp // blk
    r = srcp - b * blk
    srcp = b * blk + (r % 128) * TSB1 + r // 128
    # groups: (half-table, row parity) — parity selects which half of the
    # gathered 256-byte row pair is live; it also spreads each core's
    # self-loop band over both groups of its half.
    half = srcp // (2 * GSZ)
    w = srcp - half * (2 * GSZ)
    grp = half * 2 + (w & 1)
    idx16 = (w >> 1).astype(np.int16)     # pair index within the half

    key = ((core * NT + tl) * NG + grp)
    order = np.argsort(key * np.int64(NPAD) + srcp, kind="stable")
    key_s = key[order]
    idx_s = idx16[order]
    dloc_s = dloc[order]

    cnt = np.bincount(key, minlength=NCORES * NT * NG).reshape(NCORES, NT, NG)
    C = ((cnt.max(axis=0) + 127) // 128).astype(np.int64)      # [NT, NG] chunks
    starts = np.zeros(NCORES * NT * NG + 1, dtype=np.int64)
    np.cumsum(cnt.reshape(-1), out=starts[1:])

    # superblock partition of the 98 tiles
    sbs = [list(range(s, min(s + TSB3, NT))) for s in range(0, NT, TSB3)]
    # stream layout: for sb: for g: for t in sb: C[t,g] chunks of 128 edges
    col_of = np.zeros((NT, NG), dtype=np.int64)   # chunk column of (t, g)
    sb_meta = []
    col = 0
    for tiles in sbs:
        colbase = col
        Ls = []
        goffs = []
        for g in range(NG):
            goffs.append(col - colbase)
            for t in tiles:
                col_of[t, g] = col
                col += C[t, g]
            Ls.append(int(128 * sum(C[t, g] for t in tiles)))
        sb_meta.append(dict(tiles=tiles, colbase=int(colbase),
                            totc=int(col - colbase), L=Ls, goff=goffs))
    TOTC = int(col)
    TOT = TOTC * 128

    import ml_dtypes
    bf16 = ml_dtypes.bfloat16
    idx_streams, dloc_streams, dinv_rows = [], [], []
    for c in range(NCORES):
        si = np.zeros(TOT, dtype=np.int16)
        sd = np.full(TOT, -1.0, dtype=np.float32)
        for t in range(NT):
            for g in range(NG):
                k = (c * NT + t) * NG + g
                n = cnt[c, t, g]
                if n == 0:
                    continue
                a = starts[k]
                o = col_of[t, g] * 128
                si[o:o + n] = idx_s[a:a + n]
                sd[o:o + n] = dloc_s[a:a + n]
        idx_streams.append(np.tile(si.reshape(-1, 16).T, (8, 1)))      # [128, TOT/16]
        dloc_streams.append(
            np.ascontiguousarray(sd.reshape(-1, 128).T).astype(bf16))  # [128, TOTC]
        dvr = np.zeros((1, SPAD), dtype=np.float32)
        dvr[0, :SHARD] = dinv[c * SHARD:(c + 1) * SHARD]
        dinv_rows.append(dvr.astype(bf16))
    meta = dict(C=C, sb_meta=sb_meta, TOTC=TOTC, TOT=TOT)
    return idx_streams, dloc_streams, dinv_rows, meta, dinv


def _build_nc(meta):
    import concourse.bacc as bacc
    import concourse.mybir as mybir
    import concourse.tile as tile
    from concourse import library_config

    f32 = mybir.dt.float32
    bf16 = mybir.dt.bfloat16
    i16 = mybir.dt.int16
    AF = mybir.ActivationFunctionType
    ALU = mybir.AluOpType
    TOTC, TOT = meta["TOTC"], meta["TOT"]
    C, sb_meta = meta["C"], meta["sb_meta"]

    nc = bacc.Bacc("TRN2", num_swdge_queues=4)
    xT = nc.dram_tensor("xT", [F, NPAD], bf16, kind="ExternalInput")
    wg = nc.dram_tensor("wg", [F, H], bf16, kind="ExternalInput")
    w1 = nc.dram_tensor("w1", [64, 32], bf16, kind="ExternalInput")
    w2 = nc.dram_tensor("w2", [32, 16], bf16, kind="ExternalInput")
    w3 = nc.dram_tensor("w3", [16, 4], bf16, kind="ExternalInput")
    bg = nc.dram_tensor("bg", [64, 1], f32, kind="ExternalInput")
    b1 = nc.dram_tensor("b1", [32, 1], f32, kind="ExternalInput")
    b2 = nc.dram_tensor("b2", [16, 1], f32, kind="ExternalInput")
    b3r = nc.dram_tensor("b3r", [1, 4], bf16, kind="ExternalInput")
    iotam = nc.dram_tensor("iotam", [128, 128], bf16, kind="ExternalInput")
    onesr = nc.dram_tensor("onesr", [1, 128], bf16, kind="ExternalInput")
    idxT = nc.dram_tensor("idx", [128, TOT // 16], i16, kind="ExternalInput")
    dlocT = nc.dram_tensor("dloc", [128, TOTC], bf16, kind="ExternalInput")
    dvT = nc.dram_tensor("dv", [1, SPAD], bf16, kind="ExternalInput")
    outT = nc.dram_tensor("out", [SPAD, NCLS], f32, kind="ExternalOutput")

    NT1 = NPAD // 128  # 784 phase-1 tiles
    sb1 = [list(range(s, min(s + TSB1, NT1))) for s in range(0, NT1, TSB1)]
    # per-pass (groups 0-1 / groups 2-3) chunk-count maxima for tile sizing
    maxc0 = max(m["goff"][2] for m in sb_meta)
    maxc1 = max(m["totc"] - m["goff"][2] for m in sb_meta)
    maxc = max(maxc0, maxc1)

    with tile.TileContext(nc) as tc:
        with tc.tile_pool(name="const", bufs=1) as cp, \
             tc.tile_pool(name="dram", bufs=1, space="DRAM") as dram:
            # each half stored as row pairs: [25088 pairs, 128] bf16
            h01 = dram.tile([GSZ, 2 * H], bf16, tag="h01")
            h23 = dram.tile([GSZ, 2 * H], bf16, tag="h23")
            nc.gpsimd.load_library(library_config.mlp)

            wg0 = cp.tile([128, H], bf16, tag="wg0")
            wg1 = cp.tile([128, H], bf16, tag="wg1")
            nc.sync.dma_start(wg0[:], wg[0:128, :])
            nc.sync.dma_start(wg1[:], wg[128:256, :])
            w1s = cp.tile([64, 32], bf16, tag="w1s")
            w2s = cp.tile([32, 16], bf16, tag="w2s")
            w3s = cp.tile([16, 4], bf16, tag="w3s")
            bgs = cp.tile([64, 1], f32, tag="bgs")
            b1s = cp.tile([32, 1], f32, tag="b1s")
            b2s = cp.tile([16, 1], f32, tag="b2s")
            b3s = cp.tile([1, 4], bf16, tag="b3s")
            iots = cp.tile([128, 128], bf16, tag="iots")
            ones = cp.tile([1, 128], bf16, tag="ones")
            for t_, d_ in ((w1s, w1), (w2s, w2), (w3s, w3), (bgs, bg),
                           (b1s, b1), (b2s, b2), (b3s, b3r), (iots, iotam),
                           (ones, onesr)):
                nc.sync.dma_start(t_[:], d_[:, :])

            # All pools stay open so phase 1 overlaps the pass-0 gathers
            # (closing/reusing SBUF zones would add false dependencies).
            with tc.tile_pool(name="p1", bufs=2) as p1p, \
                 tc.tile_pool(name="ps1", bufs=2, space="PSUM") as ps1, \
                 tc.tile_pool(name="p3", bufs=3) as p3p, \
                 tc.tile_pool(name="gb", bufs=2) as gbp, \
                 tc.tile_pool(name="sp", bufs=5) as sp, \
                 tc.tile_pool(name="ep", bufs=3) as ep, \
                 tc.tile_pool(name="oa", bufs=1) as oap, \
                 tc.tile_pool(name="agg", bufs=3, space="PSUM") as aggp, \
                 tc.tile_pool(name="mlp", bufs=3, space="PSUM") as mlpp:
                # -------- phase 1: h = x @ W_gcn, halves written in order ---
                nhalf = len(sb1) // 2
                for bi, tiles in enumerate(sb1):
                    T = len(tiles)
                    t0 = tiles[0]
                    xt0 = p1p.tile([128, TSB1 * 128], bf16, tag="xt0")
                    xt1 = p1p.tile([128, TSB1 * 128], bf16, tag="xt1")
                    nc.sync.dma_start(
                        xt0[:, :T * 128], xT[0:128, t0 * 128:(t0 + T) * 128])
                    nc.sync.dma_start(
                        xt1[:, :T * 128], xT[128:256, t0 * 128:(t0 + T) * 128])
                    hsb = p1p.tile([128, TSB1 * H], bf16, tag="hsb")
                    for i in range(T):
                        ps = ps1.tile([128, H], f32, tag="hps")
                        nc.tensor.matmul(ps[:], xt0[:, i * 128:(i + 1) * 128],
                                         wg0[:], start=True, stop=False)
                        nc.tensor.matmul(ps[:], xt1[:, i * 128:(i + 1) * 128],
                                         wg1[:], start=False, stop=True)
                        nc.scalar.activation(hsb[:, i * H:(i + 1) * H], ps[:],
                                             AF.Copy)
                    hP = h01 if bi < nhalf else h23
                    r0 = (bi if bi < nhalf else bi - nhalf) * TSB1 * 128
                    # partition-major row order -> per-partition contiguous
                    # 1 KB runs: partition p holds rows r0+p*T..r0+p*T+T-1,
                    # i.e. pair rows (r0//2)+p*(T//2).. of the [GSZ,128] table
                    nc.sync.dma_start(
                        hP[r0 // 2:(r0 + T * 128) // 2, :]
                        .rearrange("(p q) f -> p q f", p=128),
                        hsb[:].rearrange("p (q f) -> p q f", q=T // 2))

                # -------- phase 3: two passes (half 0, then half 1) --------
                zall = oap.tile([128, NT, NCLS], f32, tag="zall")
                accT = oap.tile([64, NT * 128], f32, tag="accT")
                gq = 0
                for pas in (0, 1):
                    hP = h01 if pas == 0 else h23
                    gl, gh = 2 * pas, 2 * pas + 2
                    for m in sb_meta:
                        tiles = m["tiles"]
                        pco = m["goff"][gl]                  # pass col offset
                        pend = m["totc"] if pas else m["goff"][2]
                        ptc = pend - pco                     # pass chunk count
                        cb = m["colbase"] + pco              # global col base
                        idxsb = p3p.tile([128, maxc * 8], i16, tag="idx")
                        nc.sync.dma_start(idxsb[:, :ptc * 8],
                                          idxT[:, cb * 8:(cb + ptc) * 8])
                        dlsb = p3p.tile([128, maxc], bf16, tag="dl")
                        nc.sync.dma_start(dlsb[:, :ptc],
                                          dlocT[:, cb:cb + ptc])
                        gbuf = gbp.tile([128, maxc, 2 * H], bf16, tag="gbuf")
                        slist = []
                        for b0 in range(0, ptc, KS):
                            kk = min(KS, ptc - b0)
                            St = sp.tile([128, KS, 128], bf16, tag="S")
                            nc.vector.scalar_tensor_tensor(
                                St[:, :kk, :],
                                iots[:].unsqueeze(1)
                                    .broadcast_to([128, kk, 128]),
                                0.0,
                                dlsb[:, b0:b0 + kk].unsqueeze(2)
                                    .broadcast_to([128, kk, 128]),
                                op0=ALU.subtract, op1=ALU.is_equal)
                            slist.append(St)
                        for g in range(gl, gh):
                            L = m["L"][g]
                            go = m["goff"][g] - pco
                            for k in range(0, L, GCALL):
                                ni = min(GCALL, L - k)
                                c0 = go + k // 128
                                nc.gpsimd.dma_gather(
                                    gbuf[:, c0:c0 + ni // 128, :],
                                    hP[:],
                                    idxsb[:, c0 * 8:(c0 + ni // 128) * 8],
                                    ni, ni, 2 * H, queue_num=gq % 4)
                                gq += 1
                        for ti, t in enumerate(tiles):
                            agg = aggp.tile([64, 128], f32, tag="agg")
                            nch = int(C[t, gl:gh].sum())
                            done = 0
                            for g in range(gl, gh):
                                base = (m["goff"][g] - pco) + int(
                                    sum(C[tt, g] for tt in tiles[:ti]))
                                par = g & 1
                                for j in range(int(C[t, g])):
                                    pos = base + j
                                    nc.tensor.matmul(
                                        agg[:],
                                        gbuf[:, pos, par * H:(par + 1) * H],
                                        slist[pos // KS][:, pos % KS, :],
                                        start=(done == 0),
                                        stop=(done == nch - 1))
                                    done += 1
                            if pas == 0:
                                if nch == 0:
                                    nc.vector.memset(
                                        accT[:, t * 128:(t + 1) * 128], 0.0)
                                else:
                                    nc.vector.tensor_copy(
                                        accT[:, t * 128:(t + 1) * 128], agg[:])
                                continue
                            dd = ep.tile([64, 128], bf16, tag="dd")
                            nc.sync.dma_start(
                                dd[:], dvT[0:1, t * 128:(t + 1) * 128]
                                .broadcast_to([64, 128]))
                            t0p = ep.tile([64, 128], f32, tag="t0p")
                            nc.vector.tensor_add(
                                t0p[:], accT[:, t * 128:(t + 1) * 128], agg[:])
                            t0q = ep.tile([64, 128], f32, tag="t0q")
                            nc.vector.tensor_mul(t0q[:], t0p[:], dd[:])
                            t0s = ep.tile([64, 128], bf16, tag="t0")
                            nc.scalar.activation(t0s[:], t0q[:], AF.Relu,
                                                 bias=bgs[:])
                            pm1 = mlpp.tile([32, 128], f32, tag="pm")
                            nc.tensor.matmul(pm1[:], w1s[:], t0s[:],
                                             start=True, stop=True)
                            t1s = ep.tile([32, 128], bf16, tag="t1")
                            nc.scalar.activation(t1s[:], pm1[:], AF.Relu,
                                                 bias=b1s[:])
                            pm2 = mlpp.tile([16, 128], f32, tag="pm")
                            nc.tensor.matmul(pm2[:], w2s[:], t1s[:],
                                             start=True, stop=True)
                            t2s = ep.tile([16, 128], bf16, tag="t2")
                            nc.scalar.activation(t2s[:], pm2[:], AF.Relu,
                                                 bias=b2s[:])
                            pm3 = mlpp.tile([128, NCLS], f32, tag="pm")
                            nc.tensor.matmul(pm3[:], t2s[:], w3s[:],
                                             start=True, stop=False)
                            nc.tensor.matmul(pm3[:], ones[:], b3s[:],
                                             start=False, stop=True)
                            nc.scalar.activation(zall[:, t, :], pm3[:],
                                                 AF.Copy)
                # -------- batched log_softmax over all tiles --------
                nmax = oap.tile([128, NT, 1], f32, tag="nmax")
                nc.vector.tensor_reduce(nmax[:], zall[:],
                                        axis=mybir.AxisListType.X,
                                        op=ALU.max, negate=True)
                wsh = oap.tile([128, NT, NCLS], f32, tag="wsh")
                nc.vector.tensor_add(
                    wsh[:], zall[:],
                    nmax[:].broadcast_to([128, NT, NCLS]))
                esb = oap.tile([128, NT, NCLS], f32, tag="esb")
                nc.scalar.activation(esb[:], wsh[:], AF.Exp)
                ssum = oap.tile([128, NT, 1], f32, tag="ssum")
                nc.vector.tensor_reduce(ssum[:], esb[:],
                                        axis=mybir.AxisListType.X,
                                        op=ALU.add)
                lsb = oap.tile([128, NT, 1], f32, tag="lsb")
                nc.scalar.activation(lsb[:], ssum[:], AF.Ln)
                outacc = oap.tile([128, NT, NCLS], f32, tag="outacc")
                nc.vector.tensor_sub(
                    outacc[:], wsh[:],
                    lsb[:].broadcast_to([128, NT, NCLS]))
                nc.sync.dma_start(
                    outT[:, :].rearrange("(t p) c -> p t c", p=128),
                    outacc[:])
    nc.compile()
    return nc


def kernel(x, edge_index, W_gcn, b_gcn, W1, b1, W2, b2, W3, b3,
           _trace=False):
    import ml_dtypes
    from concourse.bass_utils import run_bass_kernel_spmd

    bf16 = ml_dtypes.bfloat16
    x = np.asarray(x, dtype=np.float32)
    idx_streams, dloc_streams, dinv_rows, meta, dinv = _host_prep(edge_index)
    nc = _build_nc(meta)

    # fold dinv[src] into x rows (h'[v] = dinv[v] * (x[v] @ W))
    xs = x * dinv[:, None].astype(np.float32)
    xTp = np.zeros((F, NPAD), dtype=bf16)
    xt = np.ascontiguousarray(xs.T).astype(bf16)
    for c in range(NCORES):
        xTp[:, c * SPAD:c * SPAD + SHARD] = xt[:, c * SHARD:(c + 1) * SHARD]
    common = {
        "xT": xTp,
        "wg": np.asarray(W_gcn, np.float32).astype(bf16),
        "w1": np.asarray(W1, np.float32).astype(bf16),
        "w2": np.asarray(W2, np.float32).astype(bf16),
        "w3": np.asarray(W3, np.float32).astype(bf16),
        "bg": np.asarray(b_gcn, np.float32).reshape(64, 1),
        "b1": np.asarray(b1, np.float32).reshape(32, 1),
        "b2": np.asarray(b2, np.float32).reshape(16, 1),
        "b3r": np.asarray(b3, np.float32).reshape(1, 4).astype(bf16),
        "iotam": np.tile(np.arange(128, dtype=np.float32),
                         (128, 1)).astype(bf16),
        "onesr": np.ones((1, 128), dtype=np.float32).astype(bf16),
    }
    in_maps = []
    for c in range(NCORES):
        m = dict(common)
        m["idx"] = idx_streams[c]
        m["dloc"] = dloc_streams[c]
        m["dv"] = dinv_rows[c]
        in_maps.append(m)

    res = run_bass_kernel_spmd(nc, in_maps, core_ids=list(range(NCORES)),
                               trace=_trace)
    out = np.concatenate(
        [res.results[c]["out"][:SHARD] for c in range(NCORES)], axis=0)
    if _trace:
        kernel.last_exec_time_ns = res.exec_time_ns
    return out


kernel.last_exec_time_ns = None
